# revision 2
# baseline (speedup 1.0000x reference)
"""Trainium2 Bass kernel for nn_Actor (GNN message-passing actor network), v2.

Math (per sample b):
  v  = U_w.T @ a_l ; p = W_w.T @ a_g ; q = W_w.T @ a_l
  c_z  = a_g.W_b + a_l.U_b + att_b ; c_s0 = a_g.W_b + a_l.W_b + att_b
  ymv[b,l,d] = x[b,l,d] * v[d]                (bf16)
  t[b,l]  = sum_d ymv                         (tensor_reduce over d)
  sl      = lrelu(t + G.p + c_z);  s0 = lrelu(G.(p+q) + c_s0)
  total   = s0 + sum sl;  S = sum sl
  m'[b,d] = sum_l sl*ymv = v[d] * sum_l sl*x  (bf16 mult + in-place tree)
  mSs     = [m' * (1/v), S] / total ; g_aug = [G, 1]*s0/total
  states  = relu([ [W_w.T; W_b]^T g_aug ; [U_w.T; U_b]^T mSs ])
  out     = sigmoid(l3(relu(l2(relu(l1(states))))))

Sharding: pure data parallel, batch 4096 -> 8 cores x 512.
Engine split: Act converts x->bf16 + all activation/bias ops; DVE does the two
big bf16 multiplies + reduce + tree top; GpSimd does tree tail + small scaling;
PE does transposes + all matmuls in bf16.
"""

import numpy as np
from contextlib import ExitStack

import concourse.bass as bass
import concourse.bacc as bacc
import concourse.tile as tile
from concourse import masks, mybir
from concourse.bass_utils import run_bass_kernel_spmd

FP32 = mybir.dt.float32
BF16 = mybir.dt.bfloat16
AX = mybir.AxisListType
OP = mybir.AluOpType
AF = mybir.ActivationFunctionType

B, L = 4096, 200
GD, LD, AD, H = 64, 64, 8, 32
NCORES = 8
BC = B // NCORES          # 512 samples per core
PT = 128                  # samples per tile
NT = BC // PT             # 4 tiles per core

# how many tree levels stay on DVE before handing to GpSimd (rest of levels)
DVE_TREE_LEVELS = 2
USE_TTR = False      # tensor_tensor_reduce for the G-dot products
GP_TAILS = False     # gpsimd handles tree tails + small scaling ops
INPLACE_YM = False   # m-pass multiply in place on ymv

_CACHE = {}


def build_graph(c_z: float, c_s0: float):
    nc = bacc.Bacc()
    x = nc.declare_dram_parameter("x", [BC, L, LD], FP32, isOutput=False)
    g = nc.declare_dram_parameter("g", [BC, GD], FP32, isOutput=False)
    waug = nc.declare_dram_parameter("waug", [GD + 1, H], BF16, isOutput=False)
    uaug = nc.declare_dram_parameter("uaug", [LD + 1, H], BF16, isOutput=False)
    vb16 = nc.declare_dram_parameter("vb16", [128, LD], BF16, isOutput=False)
    rinv = nc.declare_dram_parameter("rinv", [128, LD + 1], FP32, isOutput=False)
    pb = nc.declare_dram_parameter("pb", [128, GD], FP32, isOutput=False)
    pqb = nc.declare_dram_parameter("pqb", [128, GD], FP32, isOutput=False)
    l1wt = nc.declare_dram_parameter("l1wt", [GD, 256], BF16, isOutput=False)
    l1b = nc.declare_dram_parameter("l1b", [128, 2], FP32, isOutput=False)
    l2wt = nc.declare_dram_parameter("l2wt", [256, 256], BF16, isOutput=False)
    l2b = nc.declare_dram_parameter("l2b", [128, 2], FP32, isOutput=False)
    l3wt = nc.declare_dram_parameter("l3wt", [256, AD], BF16, isOutput=False)
    l3b = nc.declare_dram_parameter("l3b", [AD, 1], FP32, isOutput=False)
    out = nc.declare_dram_parameter("out", [BC, AD], FP32, isOutput=True)

    with tile.TileContext(nc) as tc, ExitStack() as ctx:
        consts = ctx.enter_context(tc.tile_pool(name="consts", bufs=1))

        ident = consts.tile([128, 128], FP32)
        masks.make_identity(nc, ident[:])
        czt = consts.tile([128, 1], FP32)
        nc.vector.memset(czt[:], float(c_z))
        cst = consts.tile([128, 1], FP32)
        nc.vector.memset(cst[:], float(c_s0))

        waug_sb = consts.tile([GD + 1, H], BF16)
        nc.scalar.dma_start(out=waug_sb[:], in_=waug[:])
        uaug_sb = consts.tile([LD + 1, H], BF16)
        nc.scalar.dma_start(out=uaug_sb[:], in_=uaug[:])
        vb_sb = consts.tile([128, 1, LD], BF16)
        nc.scalar.dma_start(out=vb_sb[:], in_=vb16[:].rearrange("p (o d) -> p o d", o=1))
        rinv_sb = consts.tile([128, LD + 1], FP32)
        nc.scalar.dma_start(out=rinv_sb[:], in_=rinv[:])
        pb_sb = consts.tile([128, GD], FP32)
        nc.scalar.dma_start(out=pb_sb[:], in_=pb[:])
        pqb_sb = consts.tile([128, GD], FP32)
        nc.scalar.dma_start(out=pqb_sb[:], in_=pqb[:])
        l1wt_sb = consts.tile([GD, 256], BF16)
        nc.scalar.dma_start(out=l1wt_sb[:], in_=l1wt[:])
        l1b_sb = consts.tile([128, 2], FP32)
        nc.scalar.dma_start(out=l1b_sb[:], in_=l1b[:])
        l2wt_a = consts.tile([128, 256], BF16)
        nc.scalar.dma_start(out=l2wt_a[:], in_=l2wt[0:128])
        l2wt_b = consts.tile([128, 256], BF16)
        nc.scalar.dma_start(out=l2wt_b[:], in_=l2wt[128:256])
        l2b_sb = consts.tile([128, 2], FP32)
        nc.scalar.dma_start(out=l2b_sb[:], in_=l2b[:])
        l3wt_a = consts.tile([128, AD], BF16)
        nc.scalar.dma_start(out=l3wt_a[:], in_=l3wt[0:128])
        l3wt_b = consts.tile([128, AD], BF16)
        nc.scalar.dma_start(out=l3wt_b[:], in_=l3wt[128:256])
        l3b_sb = consts.tile([AD, 1], FP32)
        nc.scalar.dma_start(out=l3b_sb[:], in_=l3b[:])

        NCH = 4                    # x DMA/convert chunks per tile
        LCH = L // NCH
        xfp = ctx.enter_context(tc.tile_pool(name="xfp", bufs=6))
        xbp = ctx.enter_context(tc.tile_pool(name="xbp", bufs=4))
        ymp = ctx.enter_context(tc.tile_pool(name="ymp", bufs=1))
        ymbp = ctx.enter_context(tc.tile_pool(name="ymbp", bufs=1))
        tdp = ctx.enter_context(tc.tile_pool(name="tdp", bufs=1))
        sp = ctx.enter_context(tc.tile_pool(name="sp", bufs=2))
        fp = ctx.enter_context(tc.tile_pool(name="fp", bufs=2))
        glob = ctx.enter_context(tc.tile_pool(name="glob", bufs=1))
        psA = ctx.enter_context(tc.tile_pool(name="psA", bufs=2, space="PSUM"))
        psB = ctx.enter_context(tc.tile_pool(name="psB", bufs=2, space="PSUM"))
        ps1 = ctx.enter_context(tc.tile_pool(name="ps1", bufs=1, space="PSUM"))

        st_h0 = glob.tile([2 * H, BC // 2], BF16, tag="st_h0")
        st_h1 = glob.tile([2 * H, BC // 2], BF16, tag="st_h1")
        states_halves = [st_h0, st_h1]

        BH = BC // 2

        def emit_mlp(hh):
            """MLP for one half of 256 samples; emitted right after its
            states half completes so it overlaps later tiles' streaming."""
            str_ = states_halves[hh]

            p1a = ps1.tile([128, BH], FP32, tag="p1a")
            nc.tensor.matmul(p1a[:], lhsT=l1wt_sb[:, 0:128], rhs=str_[:], start=True, stop=True)
            p1b = ps1.tile([128, BH], FP32, tag="p1b")
            nc.tensor.matmul(p1b[:], lhsT=l1wt_sb[:, 128:256], rhs=str_[:], start=True, stop=True)
            a1a = fp.tile([128, BH], BF16, tag="a1a")
            nc.scalar.activation(out=a1a[:], in_=p1a[:], func=AF.Relu,
                                 bias=l1b_sb[:, 0:1], scale=1.0)
            a1b = fp.tile([128, BH], BF16, tag="a1b")
            nc.scalar.activation(out=a1b[:], in_=p1b[:], func=AF.Relu,
                                 bias=l1b_sb[:, 1:2], scale=1.0)

            p2a = ps1.tile([128, BH], FP32, tag="p1a")
            nc.tensor.matmul(p2a[:], lhsT=l2wt_a[:, 0:128], rhs=a1a[:], start=True, stop=False)
            nc.tensor.matmul(p2a[:], lhsT=l2wt_b[:, 0:128], rhs=a1b[:], start=False, stop=True)
            p2b = ps1.tile([128, BH], FP32, tag="p1b")
            nc.tensor.matmul(p2b[:], lhsT=l2wt_a[:, 128:256], rhs=a1a[:], start=True, stop=False)
            nc.tensor.matmul(p2b[:], lhsT=l2wt_b[:, 128:256], rhs=a1b[:], start=False, stop=True)
            a2a = fp.tile([128, BH], BF16, tag="a2a")
            nc.scalar.activation(out=a2a[:], in_=p2a[:], func=AF.Relu,
                                 bias=l2b_sb[:, 0:1], scale=1.0)
            a2b = fp.tile([128, BH], BF16, tag="a2b")
            nc.scalar.activation(out=a2b[:], in_=p2b[:], func=AF.Relu,
                                 bias=l2b_sb[:, 1:2], scale=1.0)

            p3 = ps1.tile([AD, BH], FP32, tag="p1a")
            nc.tensor.matmul(p3[:], lhsT=l3wt_a[:], rhs=a2a[:], start=True, stop=False)
            nc.tensor.matmul(p3[:], lhsT=l3wt_b[:], rhs=a2b[:], start=False, stop=True)
            oT = fp.tile([AD, BH], FP32, tag="oT")
            nc.scalar.activation(out=oT[:], in_=p3[:], func=AF.Sigmoid,
                                 bias=l3b_sb[:], scale=1.0)

            for q in range(2):
                b0o = hh * BH + q * PT
                pO = psA.tile([PT, AD], FP32, tag="pO")
                nc.tensor.transpose(pO[:], oT[:, q * PT:(q + 1) * PT], ident[0:AD, 0:AD])
                ob = fp.tile([PT, AD], FP32, tag="ob")
                nc.scalar.copy(out=ob[:], in_=pO[:])
                nc.sync.dma_start(out=out[b0o:b0o + PT, :], in_=ob[:])

        for it in range(NT):
            b0 = it * PT
            # ---- load x in chunks; convert fp32 -> bf16 on Act;
            #      multiply by v on DVE per chunk (frees fp32/bf16 buffers) ----
            ymv = ymp.tile([PT, L, LD], BF16, tag="ymv")
            for cchunk in range(NCH):
                ls = slice(cchunk * LCH, (cchunk + 1) * LCH)
                xf = xfp.tile([PT, LCH, LD], FP32, tag="xf")
                nc.sync.dma_start(out=xf[:], in_=x[b0:b0 + PT, ls])
                xbh = xbp.tile([PT, LCH, LD], BF16, tag="xbh")
                nc.scalar.copy(out=xbh[:], in_=xf[:])
                nc.vector.tensor_mul(
                    out=ymv[:, ls, :], in0=xbh[:],
                    in1=vb_sb[:].to_broadcast([PT, LCH, LD]))
            gt = sp.tile([PT, GD], FP32, tag="gt")
            nc.sync.dma_start(out=gt[:], in_=g[b0:b0 + PT])

            # ---- per-sample bias dots ----
            junk = sp.tile([PT, GD], FP32, tag="junk")
            bzc = sp.tile([PT, 1], FP32, tag="bzc")
            s0z = sp.tile([PT, 1], FP32, tag="s0z")
            if USE_TTR:
                nc.vector.tensor_tensor_reduce(
                    out=junk[:], in0=gt[:], in1=pb_sb[:], scale=1.0, scalar=float(c_z),
                    op0=OP.mult, op1=OP.add, accum_out=bzc[:])
                nc.vector.tensor_tensor_reduce(
                    out=junk[:], in0=gt[:], in1=pqb_sb[:], scale=1.0, scalar=float(c_s0),
                    op0=OP.mult, op1=OP.add, accum_out=s0z[:])
            else:
                bz0 = sp.tile([PT, 1], FP32, tag="bz0")
                nc.vector.tensor_mul(out=junk[:], in0=gt[:], in1=pb_sb[:])
                nc.vector.reduce_sum(out=bz0[:], in_=junk[:], axis=AX.X)
                nc.scalar.activation(out=bzc[:], in_=bz0[:], func=AF.Identity,
                                     bias=czt[:, 0:1])
                nc.vector.tensor_mul(out=junk[:], in0=gt[:], in1=pqb_sb[:])
                nc.vector.reduce_sum(out=bz0[:], in_=junk[:], axis=AX.X)
                nc.scalar.activation(out=s0z[:], in_=bz0[:], func=AF.Identity,
                                     bias=cst[:, 0:1])
            s0 = sp.tile([PT, 1], FP32, tag="s0")
            nc.scalar.activation(out=s0[:], in_=s0z[:], func=AF.Lrelu, alpha=0.01)

            # ---- t-pass: tree-fold ymv over d into t16 (DVE top, GpSimd tail) ----
            td = tdp.tile([PT, L, LD // 2], BF16, tag="td")
            nc.vector.tensor_add(out=td[:], in0=ymv[:, :, 0:LD // 2],
                                 in1=ymv[:, :, LD // 2:LD])
            gp_eng = nc.gpsimd if GP_TAILS else nc.vector
            dh = LD // 2
            lvl = 1
            while dh > 2:
                h = dh // 2
                eng = nc.vector if lvl < DVE_TREE_LEVELS else gp_eng
                eng.tensor_add(out=td[:, :, 0:h], in0=td[:, :, 0:h],
                               in1=td[:, :, h:dh])
                dh = h
                lvl += 1
            t16 = sp.tile([PT, L], FP32, tag="t16")
            gp_eng.tensor_add(
                out=t16[:].rearrange("p (l o) -> p l o", o=1),
                in0=td[:, :, 0:1], in1=td[:, :, 1:2])

            # sl = lrelu(t + bzc); S = sum(sl)  (single fused Act op, fp32 in
            # -> bf16 out so the slb2 broadcast copy is a bf16->bf16 pattern)
            mS = sp.tile([PT, LD + 1], FP32, tag="mS")
            sl = sp.tile([PT, L], BF16, tag="sl")
            nc.scalar.activation(
                out=sl[:], in_=t16[:], func=AF.Lrelu,
                bias=bzc[:], alpha=0.01, accum_out=mS[:, LD:LD + 1])

            # slb2: sl duplicated pairs in bf16 [P, L, 2]
            slb2 = sp.tile([PT, L, 2], BF16, tag="slb2")
            nc.vector.tensor_copy(
                out=slb2[:],
                in_=sl[:].rearrange("p (l o) -> p l o", o=1).to_broadcast([PT, L, 2]))

            # ---- m-pass: ym = ymv * sl, tree-fold over l ----
            if INPLACE_YM:
                ymt = ymv
            else:
                ymt = ymbp.tile([PT, L, LD], BF16, tag="ymb")
            ymi = ymv[:].rearrange("p l (o t) -> p l o t", o=LD // 2)
            ymo = ymt[:].rearrange("p l (o t) -> p l o t", o=LD // 2)
            nc.vector.tensor_mul(
                out=ymo, in0=ymi,
                in1=slb2[:].rearrange("p l (o t) -> p l o t", o=1)
                    .to_broadcast([PT, L, LD // 2, 2]))
            cur = ymt[:]
            n = L
            lvl = 0
            while n > 3:
                h = n // 2
                odd = n - 2 * h
                eng = nc.vector if lvl < DVE_TREE_LEVELS else gp_eng
                eng.tensor_add(out=cur[:, 0:h, :], in0=cur[:, 0:h, :],
                               in1=cur[:, h:2 * h, :])
                if odd:
                    eng.tensor_add(out=cur[:, 0:1, :], in0=cur[:, 0:1, :],
                                   in1=cur[:, 2 * h:n, :])
                n = h
                lvl += 1
            # final level(s) -> fp32 m' into mS[:, 0:64]
            m1 = mS[:, 0:LD].rearrange("p (o d) -> p o d", o=1)
            gp_eng.tensor_add(out=m1, in0=cur[:, 0:1, :], in1=cur[:, 1:2, :])
            if n == 3:
                gp_eng.tensor_add(out=m1, in0=m1, in1=cur[:, 2:3, :])

            # ---- normalization ----
            total = sp.tile([PT, 1], FP32, tag="total")
            nc.scalar.activation(out=total[:], in_=s0[:], func=AF.Identity,
                                 bias=mS[:, LD:LD + 1], scale=1.0)
            rc = sp.tile([PT, 1], FP32, tag="rc")
            nc.vector.reciprocal(out=rc[:], in_=total[:])

            # g_aug = [gt, 1] * n0s ; mSs = mS * rinv * rc
            g_aug = sp.tile([PT, GD + 1], FP32, tag="g_aug")
            gp_eng.tensor_mul(out=g_aug[:, GD:GD + 1], in0=s0[:], in1=rc[:])
            gp_eng.tensor_mul(
                out=g_aug[:, 0:GD], in0=gt[:],
                in1=g_aug[:, GD:GD + 1].to_broadcast([PT, GD]))
            mSs = sp.tile([PT, LD + 1], FP32, tag="mSs")
            gp_eng.tensor_mul(out=mSs[:], in0=mS[:], in1=rinv_sb[:])
            gp_eng.tensor_mul(out=mSs[:], in0=mSs[:],
                              in1=rc[:].to_broadcast([PT, LD + 1]))

            # ---- transposes + phase-A matmuls (bf16) ----
            pG = psA.tile([GD + 1, PT], FP32, tag="pG")
            nc.tensor.transpose(pG[:], g_aug[:], ident[:])
            gTs = fp.tile([GD + 1, PT], BF16, tag="gTs")
            nc.scalar.copy(out=gTs[:], in_=pG[:])

            pM = psA.tile([LD + 1, PT], FP32, tag="pG")
            nc.tensor.transpose(pM[:], mSs[:], ident[:])
            msts = fp.tile([LD + 1, PT], BF16, tag="msts")
            nc.scalar.copy(out=msts[:], in_=pM[:])

            sh = states_halves[it // 2]
            c0 = (it % 2) * PT
            pW = psB.tile([H, PT], FP32, tag="pW")
            nc.tensor.matmul(pW[:], lhsT=waug_sb[:], rhs=gTs[:], start=True, stop=True)
            nc.scalar.activation(out=sh[0:H, c0:c0 + PT], in_=pW[:], func=AF.Relu)
            pAg = psB.tile([H, PT], FP32, tag="pW")
            nc.tensor.matmul(pAg[:], lhsT=uaug_sb[:], rhs=msts[:], start=True, stop=True)
            nc.scalar.activation(out=sh[H:2 * H, c0:c0 + PT], in_=pAg[:], func=AF.Relu)

            if it % 2 == 1:
                emit_mlp(it // 2)

    nc.compile()
    return nc


def _prep(inputs):
    import ml_dtypes
    W_w = np.asarray(inputs["W_w"], np.float32)
    W_b = np.asarray(inputs["W_b"], np.float32)
    U_w = np.asarray(inputs["U_w"], np.float32)
    U_b = np.asarray(inputs["U_b"], np.float32)
    att_w = np.asarray(inputs["att_w"], np.float32)
    att_b = np.asarray(inputs["att_b"], np.float32)
    l1_w = np.asarray(inputs["l1_w"], np.float32)
    l1_b = np.asarray(inputs["l1_b"], np.float32)
    l2_w = np.asarray(inputs["l2_w"], np.float32)
    l2_b = np.asarray(inputs["l2_b"], np.float32)
    l3_w = np.asarray(inputs["l3_w"], np.float32)
    l3_b = np.asarray(inputs["l3_b"], np.float32)

    a_g, a_l = att_w[0, :H], att_w[0, H:]
    v = U_w.T @ a_l
    p = W_w.T @ a_g
    q = W_w.T @ a_l
    c_g = float(a_g @ W_b)
    c_q = float(a_l @ W_b)
    c_v = float(a_l @ U_b)
    ab = float(att_b[0])
    c_z = c_g + c_v + ab
    c_s0 = c_g + c_q + ab

    bf = ml_dtypes.bfloat16
    # 1/v computed against the bf16-rounded v actually used on device
    v16 = v.astype(bf)
    v16f = v16.astype(np.float32)
    v16f = np.where(np.abs(v16f) < 1e-20, 1e-20, v16f)
    rinv_row = np.concatenate([1.0 / v16f, [1.0]]).astype(np.float32)

    consts = dict(
        waug=np.ascontiguousarray(np.vstack([W_w.T, W_b[None, :]]).astype(bf)),
        uaug=np.ascontiguousarray(np.vstack([U_w.T, U_b[None, :]]).astype(bf)),
        vb16=np.ascontiguousarray(np.broadcast_to(v16, (128, LD))),
        rinv=np.ascontiguousarray(np.broadcast_to(rinv_row, (128, LD + 1))),
        pb=np.ascontiguousarray(np.broadcast_to(p, (128, GD)).astype(np.float32)),
        pqb=np.ascontiguousarray(np.broadcast_to(p + q, (128, GD)).astype(np.float32)),
        l1wt=np.ascontiguousarray(l1_w.T.astype(bf)),
        l1b=np.ascontiguousarray(l1_b.reshape(2, 128).T.astype(np.float32)),
        l2wt=np.ascontiguousarray(l2_w.T.astype(bf)),
        l2b=np.ascontiguousarray(l2_b.reshape(2, 128).T.astype(np.float32)),
        l3wt=np.ascontiguousarray(l3_w.T.astype(bf)),
        l3b=np.ascontiguousarray(l3_b[:, None].astype(np.float32)),
    )
    return consts, c_z, c_s0


def _get_graph_and_consts(inputs):
    consts, c_z, c_s0 = _prep(inputs)
    key = (c_z, c_s0)
    if key not in _CACHE:
        _CACHE[key] = build_graph(c_z, c_s0)
    return _CACHE[key], consts


def kernel(**inputs) -> np.ndarray:
    nc, consts = _get_graph_and_consts(inputs)
    gs = np.ascontiguousarray(np.asarray(inputs["global_states"], np.float32))
    ls = np.ascontiguousarray(np.asarray(inputs["local_states"], np.float32))
    in_maps = []
    for i in range(NCORES):
        m = dict(consts)
        m["x"] = np.ascontiguousarray(ls[i * BC:(i + 1) * BC])
        m["g"] = np.ascontiguousarray(gs[i * BC:(i + 1) * BC])
        in_maps.append(m)
    res = run_bass_kernel_spmd(nc, in_maps, list(range(NCORES)))
    outs = [res.results[i]["out"] for i in range(NCORES)]
    return np.concatenate(outs, axis=0).astype(np.float32)


# revision 3
# speedup vs baseline: 1.1715x; 1.1715x over previous
"""Trainium2 Bass kernel for nn_Actor (GNN message-passing actor network), v2.

Math (per sample b):
  v  = U_w.T @ a_l ; p = W_w.T @ a_g ; q = W_w.T @ a_l
  c_z  = a_g.W_b + a_l.U_b + att_b ; c_s0 = a_g.W_b + a_l.W_b + att_b
  ymv[b,l,d] = x[b,l,d] * v[d]                (bf16)
  t[b,l]  = sum_d ymv                         (tensor_reduce over d)
  sl      = lrelu(t + G.p + c_z);  s0 = lrelu(G.(p+q) + c_s0)
  total   = s0 + sum sl;  S = sum sl
  m'[b,d] = sum_l sl*ymv = v[d] * sum_l sl*x  (bf16 mult + in-place tree)
  mSs     = [m' * (1/v), S] / total ; g_aug = [G, 1]*s0/total
  states  = relu([ [W_w.T; W_b]^T g_aug ; [U_w.T; U_b]^T mSs ])
  out     = sigmoid(l3(relu(l2(relu(l1(states))))))

Sharding: pure data parallel, batch 4096 -> 8 cores x 512.
Engine split: Act converts x->bf16 + all activation/bias ops; DVE does the two
big bf16 multiplies + reduce + tree top; GpSimd does tree tail + small scaling;
PE does transposes + all matmuls in bf16.
"""

import numpy as np
from contextlib import ExitStack

import concourse.bass as bass
import concourse.bacc as bacc
import concourse.tile as tile
from concourse import masks, mybir
from concourse.bass_utils import run_bass_kernel_spmd

FP32 = mybir.dt.float32
BF16 = mybir.dt.bfloat16
AX = mybir.AxisListType
OP = mybir.AluOpType
AF = mybir.ActivationFunctionType

B, L = 4096, 200
GD, LD, AD, H = 64, 64, 8, 32
NCORES = 8
BC = B // NCORES          # 512 samples per core
PT = 128                  # samples per tile
NT = BC // PT             # 4 tiles per core

# how many tree levels stay on DVE before handing to GpSimd (rest of levels)
DVE_TREE_LEVELS = 2
USE_TTR = False      # tensor_tensor_reduce for the G-dot products
GP_TAILS = False     # gpsimd handles tree tails + small scaling ops
INPLACE_YM = True    # m-pass multiply in place on ymv

_CACHE = {}


def build_graph(c_z: float, c_s0: float):
    nc = bacc.Bacc()
    x = nc.declare_dram_parameter("x", [BC, L, LD], FP32, isOutput=False)
    g = nc.declare_dram_parameter("g", [BC, GD], FP32, isOutput=False)
    waug = nc.declare_dram_parameter("waug", [GD + 1, H], BF16, isOutput=False)
    uaug = nc.declare_dram_parameter("uaug", [LD + 1, H], BF16, isOutput=False)
    vb16 = nc.declare_dram_parameter("vb16", [128, LD], BF16, isOutput=False)
    rinv = nc.declare_dram_parameter("rinv", [128, LD + 1], FP32, isOutput=False)
    pb = nc.declare_dram_parameter("pb", [128, GD], FP32, isOutput=False)
    pqb = nc.declare_dram_parameter("pqb", [128, GD], FP32, isOutput=False)
    l1wt = nc.declare_dram_parameter("l1wt", [GD, 256], BF16, isOutput=False)
    l1b = nc.declare_dram_parameter("l1b", [128, 2], FP32, isOutput=False)
    l2wt = nc.declare_dram_parameter("l2wt", [256, 256], BF16, isOutput=False)
    l2b = nc.declare_dram_parameter("l2b", [128, 2], FP32, isOutput=False)
    l3wt = nc.declare_dram_parameter("l3wt", [256, AD], BF16, isOutput=False)
    l3b = nc.declare_dram_parameter("l3b", [AD, 1], FP32, isOutput=False)
    out = nc.declare_dram_parameter("out", [BC, AD], FP32, isOutput=True)

    with tile.TileContext(nc) as tc, ExitStack() as ctx:
        consts = ctx.enter_context(tc.tile_pool(name="consts", bufs=1))

        ident = consts.tile([128, 128], FP32)
        masks.make_identity(nc, ident[:])
        czt = consts.tile([128, 1], FP32)
        nc.vector.memset(czt[:], float(c_z))
        cst = consts.tile([128, 1], FP32)
        nc.vector.memset(cst[:], float(c_s0))

        waug_sb = consts.tile([GD + 1, H], BF16)
        nc.sync.dma_start(out=waug_sb[:], in_=waug[:])
        uaug_sb = consts.tile([LD + 1, H], BF16)
        nc.sync.dma_start(out=uaug_sb[:], in_=uaug[:])
        vb_sb = consts.tile([128, 1, LD], BF16)
        nc.sync.dma_start(out=vb_sb[:], in_=vb16[:].rearrange("p (o d) -> p o d", o=1))
        rinv_sb = consts.tile([128, LD + 1], FP32)
        nc.sync.dma_start(out=rinv_sb[:], in_=rinv[:])
        pb_sb = consts.tile([128, GD], FP32)
        nc.sync.dma_start(out=pb_sb[:], in_=pb[:])
        pqb_sb = consts.tile([128, GD], FP32)
        nc.sync.dma_start(out=pqb_sb[:], in_=pqb[:])
        l1wt_sb = consts.tile([GD, 256], BF16)
        nc.sync.dma_start(out=l1wt_sb[:], in_=l1wt[:])
        l1b_sb = consts.tile([128, 2], FP32)
        nc.sync.dma_start(out=l1b_sb[:], in_=l1b[:])
        l2wt_a = consts.tile([128, 256], BF16)
        nc.sync.dma_start(out=l2wt_a[:], in_=l2wt[0:128])
        l2wt_b = consts.tile([128, 256], BF16)
        nc.sync.dma_start(out=l2wt_b[:], in_=l2wt[128:256])
        l2b_sb = consts.tile([128, 2], FP32)
        nc.sync.dma_start(out=l2b_sb[:], in_=l2b[:])
        l3wt_a = consts.tile([128, AD], BF16)
        nc.sync.dma_start(out=l3wt_a[:], in_=l3wt[0:128])
        l3wt_b = consts.tile([128, AD], BF16)
        nc.sync.dma_start(out=l3wt_b[:], in_=l3wt[128:256])
        l3b_sb = consts.tile([AD, 1], FP32)
        nc.sync.dma_start(out=l3b_sb[:], in_=l3b[:])

        NCH = 4                    # x DMA/convert chunks per tile
        LCH = L // NCH
        xfp = ctx.enter_context(tc.tile_pool(name="xfp", bufs=6))
        xbp = ctx.enter_context(tc.tile_pool(name="xbp", bufs=4))
        ymp = ctx.enter_context(tc.tile_pool(name="ymp", bufs=2))
        ymbp = ctx.enter_context(tc.tile_pool(name="ymbp", bufs=1))
        tdp = ctx.enter_context(tc.tile_pool(name="tdp", bufs=1))
        sp = ctx.enter_context(tc.tile_pool(name="sp", bufs=2))
        fp = ctx.enter_context(tc.tile_pool(name="fp", bufs=2))
        glob = ctx.enter_context(tc.tile_pool(name="glob", bufs=1))
        psA = ctx.enter_context(tc.tile_pool(name="psA", bufs=2, space="PSUM"))
        psB = ctx.enter_context(tc.tile_pool(name="psB", bufs=2, space="PSUM"))
        ps1 = ctx.enter_context(tc.tile_pool(name="ps1", bufs=1, space="PSUM"))

        st_h0 = glob.tile([2 * H, BC // 2], BF16, tag="st_h0")
        st_h1 = glob.tile([2 * H, BC // 2], BF16, tag="st_h1")
        states_halves = [st_h0, st_h1]

        BH = BC // 2

        def emit_mlp(hh):
            """MLP for one half of 256 samples; emitted right after its
            states half completes so it overlaps later tiles' streaming."""
            str_ = states_halves[hh]

            p1a = ps1.tile([128, BH], FP32, tag="p1a")
            nc.tensor.matmul(p1a[:], lhsT=l1wt_sb[:, 0:128], rhs=str_[:], start=True, stop=True)
            p1b = ps1.tile([128, BH], FP32, tag="p1b")
            nc.tensor.matmul(p1b[:], lhsT=l1wt_sb[:, 128:256], rhs=str_[:], start=True, stop=True)
            a1a = fp.tile([128, BH], BF16, tag="a1a")
            nc.scalar.activation(out=a1a[:], in_=p1a[:], func=AF.Relu,
                                 bias=l1b_sb[:, 0:1], scale=1.0)
            a1b = fp.tile([128, BH], BF16, tag="a1b")
            nc.scalar.activation(out=a1b[:], in_=p1b[:], func=AF.Relu,
                                 bias=l1b_sb[:, 1:2], scale=1.0)

            p2a = ps1.tile([128, BH], FP32, tag="p1a")
            nc.tensor.matmul(p2a[:], lhsT=l2wt_a[:, 0:128], rhs=a1a[:], start=True, stop=False)
            nc.tensor.matmul(p2a[:], lhsT=l2wt_b[:, 0:128], rhs=a1b[:], start=False, stop=True)
            p2b = ps1.tile([128, BH], FP32, tag="p1b")
            nc.tensor.matmul(p2b[:], lhsT=l2wt_a[:, 128:256], rhs=a1a[:], start=True, stop=False)
            nc.tensor.matmul(p2b[:], lhsT=l2wt_b[:, 128:256], rhs=a1b[:], start=False, stop=True)
            a2a = fp.tile([128, BH], BF16, tag="a2a")
            nc.scalar.activation(out=a2a[:], in_=p2a[:], func=AF.Relu,
                                 bias=l2b_sb[:, 0:1], scale=1.0)
            a2b = fp.tile([128, BH], BF16, tag="a2b")
            nc.scalar.activation(out=a2b[:], in_=p2b[:], func=AF.Relu,
                                 bias=l2b_sb[:, 1:2], scale=1.0)

            p3 = ps1.tile([AD, BH], FP32, tag="p1a")
            nc.tensor.matmul(p3[:], lhsT=l3wt_a[:], rhs=a2a[:], start=True, stop=False)
            nc.tensor.matmul(p3[:], lhsT=l3wt_b[:], rhs=a2b[:], start=False, stop=True)
            oT = fp.tile([AD, BH], FP32, tag="oT")
            nc.scalar.activation(out=oT[:], in_=p3[:], func=AF.Sigmoid,
                                 bias=l3b_sb[:], scale=1.0)

            for q in range(2):
                b0o = hh * BH + q * PT
                pO = psA.tile([PT, AD], FP32, tag="pO")
                nc.tensor.transpose(pO[:], oT[:, q * PT:(q + 1) * PT], ident[0:AD, 0:AD])
                ob = fp.tile([PT, AD], FP32, tag="ob")
                nc.scalar.copy(out=ob[:], in_=pO[:])
                nc.sync.dma_start(out=out[b0o:b0o + PT, :], in_=ob[:])

        def emit_load(it):
            """DMA x in chunks; convert fp32->bf16 on Act; multiply by v on
            DVE per chunk. Returns (ymv, gt) for the tile."""
            b0 = it * PT
            ymv = ymp.tile([PT, L, LD], BF16, tag="ymv")
            for cchunk in range(NCH):
                ls = slice(cchunk * LCH, (cchunk + 1) * LCH)
                xf = xfp.tile([PT, LCH, LD], FP32, tag="xf")
                nc.sync.dma_start(out=xf[:], in_=x[b0:b0 + PT, ls])
                xbh = xbp.tile([PT, LCH, LD], BF16, tag="xbh")
                nc.scalar.copy(out=xbh[:], in_=xf[:])
                nc.vector.tensor_mul(
                    out=ymv[:, ls, :], in0=xbh[:],
                    in1=vb_sb[:].to_broadcast([PT, LCH, LD]))
            gt = sp.tile([PT, GD], FP32, tag="gt")
            nc.sync.dma_start(out=gt[:], in_=g[b0:b0 + PT])
            return ymv, gt

        loaded = emit_load(0)
        for it in range(NT):
            b0 = it * PT
            ymv, gt = loaded

            # ---- per-sample bias dots ----
            junk = sp.tile([PT, GD], FP32, tag="junk")
            bzc = sp.tile([PT, 1], FP32, tag="bzc")
            s0z = sp.tile([PT, 1], FP32, tag="s0z")
            if USE_TTR:
                nc.vector.tensor_tensor_reduce(
                    out=junk[:], in0=gt[:], in1=pb_sb[:], scale=1.0, scalar=float(c_z),
                    op0=OP.mult, op1=OP.add, accum_out=bzc[:])
                nc.vector.tensor_tensor_reduce(
                    out=junk[:], in0=gt[:], in1=pqb_sb[:], scale=1.0, scalar=float(c_s0),
                    op0=OP.mult, op1=OP.add, accum_out=s0z[:])
            else:
                bz0 = sp.tile([PT, 1], FP32, tag="bz0")
                nc.vector.tensor_mul(out=junk[:], in0=gt[:], in1=pb_sb[:])
                nc.vector.reduce_sum(out=bz0[:], in_=junk[:], axis=AX.X)
                nc.scalar.activation(out=bzc[:], in_=bz0[:], func=AF.Identity,
                                     bias=czt[:, 0:1])
                nc.vector.tensor_mul(out=junk[:], in0=gt[:], in1=pqb_sb[:])
                nc.vector.reduce_sum(out=bz0[:], in_=junk[:], axis=AX.X)
                nc.scalar.activation(out=s0z[:], in_=bz0[:], func=AF.Identity,
                                     bias=cst[:, 0:1])
            s0 = sp.tile([PT, 1], FP32, tag="s0")
            nc.scalar.activation(out=s0[:], in_=s0z[:], func=AF.Lrelu, alpha=0.01)

            # ---- t-pass: tree-fold ymv over d into t16 (DVE top, GpSimd tail) ----
            td = tdp.tile([PT, L, LD // 2], BF16, tag="td")
            nc.vector.tensor_add(out=td[:], in0=ymv[:, :, 0:LD // 2],
                                 in1=ymv[:, :, LD // 2:LD])
            gp_eng = nc.gpsimd if GP_TAILS else nc.vector
            dh = LD // 2
            lvl = 1
            while dh > 2:
                h = dh // 2
                eng = nc.vector if lvl < DVE_TREE_LEVELS else gp_eng
                eng.tensor_add(out=td[:, :, 0:h], in0=td[:, :, 0:h],
                               in1=td[:, :, h:dh])
                dh = h
                lvl += 1
            t16 = sp.tile([PT, L], FP32, tag="t16")
            gp_eng.tensor_add(
                out=t16[:].rearrange("p (l o) -> p l o", o=1),
                in0=td[:, :, 0:1], in1=td[:, :, 1:2])

            # ---- software pipelining: enqueue next tile's load NOW, so the
            # Act queue does conversions (and DVE the v-mults) while this
            # tile's t16 -> sl -> slb2 cross-engine round-trip is in flight.
            if it + 1 < NT:
                loaded = emit_load(it + 1)

            # sl = lrelu(t + bzc); S = sum(sl)  (single fused Act op, fp32 in
            # -> bf16 out so the slb2 broadcast copy is a bf16->bf16 pattern)
            mS = sp.tile([PT, LD + 1], FP32, tag="mS")
            sl = sp.tile([PT, L], BF16, tag="sl")
            nc.scalar.activation(
                out=sl[:], in_=t16[:], func=AF.Lrelu,
                bias=bzc[:], alpha=0.01, accum_out=mS[:, LD:LD + 1])

            # slb2: sl duplicated pairs in bf16 [P, L, 2]
            slb2 = sp.tile([PT, L, 2], BF16, tag="slb2")
            nc.vector.tensor_copy(
                out=slb2[:],
                in_=sl[:].rearrange("p (l o) -> p l o", o=1).to_broadcast([PT, L, 2]))

            # ---- m-pass: ym = ymv * sl, tree-fold over l ----
            if INPLACE_YM:
                ymt = ymv
            else:
                ymt = ymbp.tile([PT, L, LD], BF16, tag="ymb")
            ymi = ymv[:].rearrange("p l (o t) -> p l o t", o=LD // 2)
            ymo = ymt[:].rearrange("p l (o t) -> p l o t", o=LD // 2)
            nc.vector.tensor_mul(
                out=ymo, in0=ymi,
                in1=slb2[:].rearrange("p l (o t) -> p l o t", o=1)
                    .to_broadcast([PT, L, LD // 2, 2]))
            cur = ymt[:]
            n = L
            lvl = 0
            while n > 3:
                h = n // 2
                odd = n - 2 * h
                eng = nc.vector if lvl < DVE_TREE_LEVELS else gp_eng
                eng.tensor_add(out=cur[:, 0:h, :], in0=cur[:, 0:h, :],
                               in1=cur[:, h:2 * h, :])
                if odd:
                    eng.tensor_add(out=cur[:, 0:1, :], in0=cur[:, 0:1, :],
                                   in1=cur[:, 2 * h:n, :])
                n = h
                lvl += 1
            # final level(s) -> fp32 m' into mS[:, 0:64]
            m1 = mS[:, 0:LD].rearrange("p (o d) -> p o d", o=1)
            gp_eng.tensor_add(out=m1, in0=cur[:, 0:1, :], in1=cur[:, 1:2, :])
            if n == 3:
                gp_eng.tensor_add(out=m1, in0=m1, in1=cur[:, 2:3, :])

            # ---- normalization ----
            total = sp.tile([PT, 1], FP32, tag="total")
            nc.vector.tensor_add(out=total[:], in0=s0[:], in1=mS[:, LD:LD + 1])
            rc = sp.tile([PT, 1], FP32, tag="rc")
            nc.vector.reciprocal(out=rc[:], in_=total[:])

            # g_aug = [gt, 1] * n0s ; mSs = mS * rinv * rc
            g_aug = sp.tile([PT, GD + 1], FP32, tag="g_aug")
            gp_eng.tensor_mul(out=g_aug[:, GD:GD + 1], in0=s0[:], in1=rc[:])
            gp_eng.tensor_mul(
                out=g_aug[:, 0:GD], in0=gt[:],
                in1=g_aug[:, GD:GD + 1].to_broadcast([PT, GD]))
            mSs = sp.tile([PT, LD + 1], FP32, tag="mSs")
            gp_eng.tensor_mul(out=mSs[:], in0=mS[:], in1=rinv_sb[:])
            gp_eng.tensor_mul(out=mSs[:], in0=mSs[:],
                              in1=rc[:].to_broadcast([PT, LD + 1]))

            # ---- transposes + phase-A matmuls (bf16) ----
            pG = psA.tile([GD + 1, PT], FP32, tag="pG")
            nc.tensor.transpose(pG[:], g_aug[:], ident[:])
            gTs = fp.tile([GD + 1, PT], BF16, tag="gTs")
            nc.scalar.copy(out=gTs[:], in_=pG[:])

            pM = psA.tile([LD + 1, PT], FP32, tag="pG")
            nc.tensor.transpose(pM[:], mSs[:], ident[:])
            msts = fp.tile([LD + 1, PT], BF16, tag="msts")
            nc.scalar.copy(out=msts[:], in_=pM[:])

            sh = states_halves[it // 2]
            c0 = (it % 2) * PT
            pW = psB.tile([H, PT], FP32, tag="pW")
            nc.tensor.matmul(pW[:], lhsT=waug_sb[:], rhs=gTs[:], start=True, stop=True)
            nc.scalar.activation(out=sh[0:H, c0:c0 + PT], in_=pW[:], func=AF.Relu)
            pAg = psB.tile([H, PT], FP32, tag="pW")
            nc.tensor.matmul(pAg[:], lhsT=uaug_sb[:], rhs=msts[:], start=True, stop=True)
            nc.scalar.activation(out=sh[H:2 * H, c0:c0 + PT], in_=pAg[:], func=AF.Relu)

            if it % 2 == 1:
                emit_mlp(it // 2)

    nc.compile()
    return nc


def _prep(inputs):
    import ml_dtypes
    W_w = np.asarray(inputs["W_w"], np.float32)
    W_b = np.asarray(inputs["W_b"], np.float32)
    U_w = np.asarray(inputs["U_w"], np.float32)
    U_b = np.asarray(inputs["U_b"], np.float32)
    att_w = np.asarray(inputs["att_w"], np.float32)
    att_b = np.asarray(inputs["att_b"], np.float32)
    l1_w = np.asarray(inputs["l1_w"], np.float32)
    l1_b = np.asarray(inputs["l1_b"], np.float32)
    l2_w = np.asarray(inputs["l2_w"], np.float32)
    l2_b = np.asarray(inputs["l2_b"], np.float32)
    l3_w = np.asarray(inputs["l3_w"], np.float32)
    l3_b = np.asarray(inputs["l3_b"], np.float32)

    a_g, a_l = att_w[0, :H], att_w[0, H:]
    v = U_w.T @ a_l
    p = W_w.T @ a_g
    q = W_w.T @ a_l
    c_g = float(a_g @ W_b)
    c_q = float(a_l @ W_b)
    c_v = float(a_l @ U_b)
    ab = float(att_b[0])
    c_z = c_g + c_v + ab
    c_s0 = c_g + c_q + ab

    bf = ml_dtypes.bfloat16
    # 1/v computed against the bf16-rounded v actually used on device
    v16 = v.astype(bf)
    v16f = v16.astype(np.float32)
    v16f = np.where(np.abs(v16f) < 1e-20, 1e-20, v16f)
    rinv_row = np.concatenate([1.0 / v16f, [1.0]]).astype(np.float32)

    consts = dict(
        waug=np.ascontiguousarray(np.vstack([W_w.T, W_b[None, :]]).astype(bf)),
        uaug=np.ascontiguousarray(np.vstack([U_w.T, U_b[None, :]]).astype(bf)),
        vb16=np.ascontiguousarray(np.broadcast_to(v16, (128, LD))),
        rinv=np.ascontiguousarray(np.broadcast_to(rinv_row, (128, LD + 1))),
        pb=np.ascontiguousarray(np.broadcast_to(p, (128, GD)).astype(np.float32)),
        pqb=np.ascontiguousarray(np.broadcast_to(p + q, (128, GD)).astype(np.float32)),
        l1wt=np.ascontiguousarray(l1_w.T.astype(bf)),
        l1b=np.ascontiguousarray(l1_b.reshape(2, 128).T.astype(np.float32)),
        l2wt=np.ascontiguousarray(l2_w.T.astype(bf)),
        l2b=np.ascontiguousarray(l2_b.reshape(2, 128).T.astype(np.float32)),
        l3wt=np.ascontiguousarray(l3_w.T.astype(bf)),
        l3b=np.ascontiguousarray(l3_b[:, None].astype(np.float32)),
    )
    return consts, c_z, c_s0


def _get_graph_and_consts(inputs):
    consts, c_z, c_s0 = _prep(inputs)
    key = (c_z, c_s0)
    if key not in _CACHE:
        _CACHE[key] = build_graph(c_z, c_s0)
    return _CACHE[key], consts


def kernel(**inputs) -> np.ndarray:
    nc, consts = _get_graph_and_consts(inputs)
    gs = np.ascontiguousarray(np.asarray(inputs["global_states"], np.float32))
    ls = np.ascontiguousarray(np.asarray(inputs["local_states"], np.float32))
    in_maps = []
    for i in range(NCORES):
        m = dict(consts)
        m["x"] = np.ascontiguousarray(ls[i * BC:(i + 1) * BC])
        m["g"] = np.ascontiguousarray(gs[i * BC:(i + 1) * BC])
        in_maps.append(m)
    res = run_bass_kernel_spmd(nc, in_maps, list(range(NCORES)))
    outs = [res.results[i]["out"] for i in range(NCORES)]
    return np.concatenate(outs, axis=0).astype(np.float32)


# revision 4
# speedup vs baseline: 1.1855x; 1.0119x over previous
"""Trainium2 Bass kernel for nn_Actor (GNN message-passing actor network), v2.

Math (per sample b):
  v  = U_w.T @ a_l ; p = W_w.T @ a_g ; q = W_w.T @ a_l
  c_z  = a_g.W_b + a_l.U_b + att_b ; c_s0 = a_g.W_b + a_l.W_b + att_b
  ymv[b,l,d] = x[b,l,d] * v[d]                (bf16)
  t[b,l]  = sum_d ymv                         (tensor_reduce over d)
  sl      = lrelu(t + G.p + c_z);  s0 = lrelu(G.(p+q) + c_s0)
  total   = s0 + sum sl;  S = sum sl
  m'[b,d] = sum_l sl*ymv = v[d] * sum_l sl*x  (bf16 mult + in-place tree)
  mSs     = [m' * (1/v), S] / total ; g_aug = [G, 1]*s0/total
  states  = relu([ [W_w.T; W_b]^T g_aug ; [U_w.T; U_b]^T mSs ])
  out     = sigmoid(l3(relu(l2(relu(l1(states))))))

Sharding: pure data parallel, batch 4096 -> 8 cores x 512.
Engine split: Act converts x->bf16 + all activation/bias ops; DVE does the two
big bf16 multiplies + reduce + tree top; GpSimd does tree tail + small scaling;
PE does transposes + all matmuls in bf16.
"""

import numpy as np
from contextlib import ExitStack

import concourse.bass as bass
import concourse.bacc as bacc
import concourse.tile as tile
from concourse import masks, mybir
from concourse.bass_utils import run_bass_kernel_spmd

FP32 = mybir.dt.float32
BF16 = mybir.dt.bfloat16
AX = mybir.AxisListType
OP = mybir.AluOpType
AF = mybir.ActivationFunctionType

B, L = 4096, 200
GD, LD, AD, H = 64, 64, 8, 32
NCORES = 8
BC = B // NCORES          # 512 samples per core
PT = 128                  # samples per tile
NT = BC // PT             # 4 tiles per core

# how many tree levels stay on DVE before handing to GpSimd (rest of levels)
DVE_TREE_LEVELS = 2
USE_TTR = False      # tensor_tensor_reduce for the G-dot products
GP_TAILS = False     # gpsimd handles tree tails + small scaling ops
INPLACE_YM = True    # m-pass multiply in place on ymv

_CACHE = {}


def build_graph(c_z: float, c_s0: float):
    nc = bacc.Bacc()
    x = nc.declare_dram_parameter("x", [BC, L, LD], FP32, isOutput=False)
    g = nc.declare_dram_parameter("g", [BC, GD], FP32, isOutput=False)
    waug = nc.declare_dram_parameter("waug", [GD + 1, H], BF16, isOutput=False)
    uaug = nc.declare_dram_parameter("uaug", [LD + 1, H], BF16, isOutput=False)
    vb16 = nc.declare_dram_parameter("vb16", [128, LD], BF16, isOutput=False)
    rinv = nc.declare_dram_parameter("rinv", [128, LD + 1], FP32, isOutput=False)
    pb = nc.declare_dram_parameter("pb", [128, GD], FP32, isOutput=False)
    pqb = nc.declare_dram_parameter("pqb", [128, GD], FP32, isOutput=False)
    l1wt = nc.declare_dram_parameter("l1wt", [GD, 256], BF16, isOutput=False)
    l1b = nc.declare_dram_parameter("l1b", [128, 2], FP32, isOutput=False)
    l2wt = nc.declare_dram_parameter("l2wt", [256, 256], BF16, isOutput=False)
    l2b = nc.declare_dram_parameter("l2b", [128, 2], FP32, isOutput=False)
    l3wt = nc.declare_dram_parameter("l3wt", [256, AD], BF16, isOutput=False)
    l3b = nc.declare_dram_parameter("l3b", [AD, 1], FP32, isOutput=False)
    out = nc.declare_dram_parameter("out", [BC, AD], FP32, isOutput=True)

    with tile.TileContext(nc) as tc, ExitStack() as ctx:
        consts = ctx.enter_context(tc.tile_pool(name="consts", bufs=1))

        ident = consts.tile([128, 128], FP32)
        masks.make_identity(nc, ident[:])
        czt = consts.tile([128, 1], FP32)
        nc.vector.memset(czt[:], float(c_z))
        cst = consts.tile([128, 1], FP32)
        nc.vector.memset(cst[:], float(c_s0))

        waug_sb = consts.tile([GD + 1, H], BF16)
        nc.scalar.dma_start(out=waug_sb[:], in_=waug[:])
        uaug_sb = consts.tile([LD + 1, H], BF16)
        nc.scalar.dma_start(out=uaug_sb[:], in_=uaug[:])
        vb_sb = consts.tile([128, 1, LD], BF16)
        nc.scalar.dma_start(out=vb_sb[:], in_=vb16[:].rearrange("p (o d) -> p o d", o=1))
        rinv_sb = consts.tile([128, LD + 1], FP32)
        nc.scalar.dma_start(out=rinv_sb[:], in_=rinv[:])
        pb_sb = consts.tile([128, GD], FP32)
        nc.scalar.dma_start(out=pb_sb[:], in_=pb[:])
        pqb_sb = consts.tile([128, GD], FP32)
        nc.scalar.dma_start(out=pqb_sb[:], in_=pqb[:])
        l1wt_sb = consts.tile([GD, 256], BF16)
        nc.scalar.dma_start(out=l1wt_sb[:], in_=l1wt[:])
        l1b_sb = consts.tile([128, 2], FP32)
        nc.scalar.dma_start(out=l1b_sb[:], in_=l1b[:])
        l2wt_a = consts.tile([128, 256], BF16)
        nc.scalar.dma_start(out=l2wt_a[:], in_=l2wt[0:128])
        l2wt_b = consts.tile([128, 256], BF16)
        nc.scalar.dma_start(out=l2wt_b[:], in_=l2wt[128:256])
        l2b_sb = consts.tile([128, 2], FP32)
        nc.scalar.dma_start(out=l2b_sb[:], in_=l2b[:])
        l3wt_a = consts.tile([128, AD], BF16)
        nc.scalar.dma_start(out=l3wt_a[:], in_=l3wt[0:128])
        l3wt_b = consts.tile([128, AD], BF16)
        nc.scalar.dma_start(out=l3wt_b[:], in_=l3wt[128:256])
        l3b_sb = consts.tile([AD, 1], FP32)
        nc.scalar.dma_start(out=l3b_sb[:], in_=l3b[:])

        NCH = 4                    # x DMA/convert chunks per tile
        LCH = L // NCH
        xfp = ctx.enter_context(tc.tile_pool(name="xfp", bufs=6))
        xbp = ctx.enter_context(tc.tile_pool(name="xbp", bufs=4))
        ymp = ctx.enter_context(tc.tile_pool(name="ymp", bufs=2))
        ymbp = ctx.enter_context(tc.tile_pool(name="ymbp", bufs=1))
        tdp = ctx.enter_context(tc.tile_pool(name="tdp", bufs=1))
        sp = ctx.enter_context(tc.tile_pool(name="sp", bufs=2))
        fp = ctx.enter_context(tc.tile_pool(name="fp", bufs=2))
        glob = ctx.enter_context(tc.tile_pool(name="glob", bufs=1))
        psA = ctx.enter_context(tc.tile_pool(name="psA", bufs=2, space="PSUM"))
        psB = ctx.enter_context(tc.tile_pool(name="psB", bufs=2, space="PSUM"))
        ps1 = ctx.enter_context(tc.tile_pool(name="ps1", bufs=1, space="PSUM"))

        st_h0 = glob.tile([2 * H, BC // 2], BF16, tag="st_h0")
        st_h1 = glob.tile([2 * H, BC // 2], BF16, tag="st_h1")
        states_halves = [st_h0, st_h1]

        BH = PT

        def emit_mlp(it):
            """MLP for one tile's 128 samples; emitted right after the tile's
            states columns complete so it overlaps later tiles' streaming."""
            str_ = states_halves[it // 2][:, (it % 2) * PT:(it % 2) * PT + PT]

            p1a = ps1.tile([128, BH], FP32, tag="p1a")
            nc.tensor.matmul(p1a[:], lhsT=l1wt_sb[:, 0:128], rhs=str_, start=True, stop=True)
            p1b = ps1.tile([128, BH], FP32, tag="p1b")
            nc.tensor.matmul(p1b[:], lhsT=l1wt_sb[:, 128:256], rhs=str_, start=True, stop=True)
            a1a = fp.tile([128, BH], BF16, tag="a1a")
            nc.scalar.activation(out=a1a[:], in_=p1a[:], func=AF.Relu,
                                 bias=l1b_sb[:, 0:1], scale=1.0)
            a1b = fp.tile([128, BH], BF16, tag="a1b")
            nc.scalar.activation(out=a1b[:], in_=p1b[:], func=AF.Relu,
                                 bias=l1b_sb[:, 1:2], scale=1.0)

            p2a = ps1.tile([128, BH], FP32, tag="p1a")
            nc.tensor.matmul(p2a[:], lhsT=l2wt_a[:, 0:128], rhs=a1a[:], start=True, stop=False)
            nc.tensor.matmul(p2a[:], lhsT=l2wt_b[:, 0:128], rhs=a1b[:], start=False, stop=True)
            p2b = ps1.tile([128, BH], FP32, tag="p1b")
            nc.tensor.matmul(p2b[:], lhsT=l2wt_a[:, 128:256], rhs=a1a[:], start=True, stop=False)
            nc.tensor.matmul(p2b[:], lhsT=l2wt_b[:, 128:256], rhs=a1b[:], start=False, stop=True)
            a2a = fp.tile([128, BH], BF16, tag="a2a")
            nc.scalar.activation(out=a2a[:], in_=p2a[:], func=AF.Relu,
                                 bias=l2b_sb[:, 0:1], scale=1.0)
            a2b = fp.tile([128, BH], BF16, tag="a2b")
            nc.scalar.activation(out=a2b[:], in_=p2b[:], func=AF.Relu,
                                 bias=l2b_sb[:, 1:2], scale=1.0)

            p3 = ps1.tile([AD, BH], FP32, tag="p1a")
            nc.tensor.matmul(p3[:], lhsT=l3wt_a[:], rhs=a2a[:], start=True, stop=False)
            nc.tensor.matmul(p3[:], lhsT=l3wt_b[:], rhs=a2b[:], start=False, stop=True)
            oT = fp.tile([AD, BH], FP32, tag="oT")
            nc.scalar.activation(out=oT[:], in_=p3[:], func=AF.Sigmoid,
                                 bias=l3b_sb[:], scale=1.0)

            b0o = it * PT
            pO = psA.tile([PT, AD], FP32, tag="pO")
            nc.tensor.transpose(pO[:], oT[:], ident[0:AD, 0:AD])
            ob = fp.tile([PT, AD], FP32, tag="ob")
            nc.scalar.copy(out=ob[:], in_=pO[:])
            nc.sync.dma_start(out=out[b0o:b0o + PT, :], in_=ob[:])

        def emit_load(it):
            """DMA x in chunks; convert fp32->bf16 on Act; multiply by v on
            DVE per chunk. Returns (ymv, gt) for the tile."""
            b0 = it * PT
            ymv = ymp.tile([PT, L, LD], BF16, tag="ymv")
            for cchunk in range(NCH):
                ls = slice(cchunk * LCH, (cchunk + 1) * LCH)
                xf = xfp.tile([PT, LCH, LD], FP32, tag="xf")
                nc.sync.dma_start(out=xf[:], in_=x[b0:b0 + PT, ls])
                xbh = xbp.tile([PT, LCH, LD], BF16, tag="xbh")
                nc.scalar.copy(out=xbh[:], in_=xf[:])
                nc.vector.tensor_mul(
                    out=ymv[:, ls, :], in0=xbh[:],
                    in1=vb_sb[:].to_broadcast([PT, LCH, LD]))
            gt = sp.tile([PT, GD], FP32, tag="gt")
            nc.sync.dma_start(out=gt[:], in_=g[b0:b0 + PT])
            return ymv, gt

        loaded = emit_load(0)
        for it in range(NT):
            b0 = it * PT
            ymv, gt = loaded

            # ---- per-sample bias dots ----
            junk = sp.tile([PT, GD], FP32, tag="junk")
            bzc = sp.tile([PT, 1], FP32, tag="bzc")
            s0z = sp.tile([PT, 1], FP32, tag="s0z")
            if USE_TTR:
                nc.vector.tensor_tensor_reduce(
                    out=junk[:], in0=gt[:], in1=pb_sb[:], scale=1.0, scalar=float(c_z),
                    op0=OP.mult, op1=OP.add, accum_out=bzc[:])
                nc.vector.tensor_tensor_reduce(
                    out=junk[:], in0=gt[:], in1=pqb_sb[:], scale=1.0, scalar=float(c_s0),
                    op0=OP.mult, op1=OP.add, accum_out=s0z[:])
            else:
                bz0 = sp.tile([PT, 1], FP32, tag="bz0")
                nc.vector.tensor_mul(out=junk[:], in0=gt[:], in1=pb_sb[:])
                nc.vector.reduce_sum(out=bz0[:], in_=junk[:], axis=AX.X)
                nc.scalar.activation(out=bzc[:], in_=bz0[:], func=AF.Identity,
                                     bias=czt[:, 0:1])
                nc.vector.tensor_mul(out=junk[:], in0=gt[:], in1=pqb_sb[:])
                nc.vector.reduce_sum(out=bz0[:], in_=junk[:], axis=AX.X)
                nc.scalar.activation(out=s0z[:], in_=bz0[:], func=AF.Identity,
                                     bias=cst[:, 0:1])
            s0 = sp.tile([PT, 1], FP32, tag="s0")
            nc.scalar.activation(out=s0[:], in_=s0z[:], func=AF.Lrelu, alpha=0.01)

            # ---- t-pass: tree-fold ymv over d into t16 (DVE top, GpSimd tail) ----
            td = tdp.tile([PT, L, LD // 2], BF16, tag="td")
            nc.vector.tensor_add(out=td[:], in0=ymv[:, :, 0:LD // 2],
                                 in1=ymv[:, :, LD // 2:LD])
            gp_eng = nc.gpsimd if GP_TAILS else nc.vector
            dh = LD // 2
            lvl = 1
            while dh > 2:
                h = dh // 2
                eng = nc.vector if lvl < DVE_TREE_LEVELS else gp_eng
                eng.tensor_add(out=td[:, :, 0:h], in0=td[:, :, 0:h],
                               in1=td[:, :, h:dh])
                dh = h
                lvl += 1
            t16 = sp.tile([PT, L], FP32, tag="t16")
            gp_eng.tensor_add(
                out=t16[:].rearrange("p (l o) -> p l o", o=1),
                in0=td[:, :, 0:1], in1=td[:, :, 1:2])

            # ---- software pipelining: enqueue next tile's load NOW, so the
            # Act queue does conversions (and DVE the v-mults) while this
            # tile's t16 -> sl -> slb2 cross-engine round-trip is in flight.
            if it + 1 < NT:
                loaded = emit_load(it + 1)

            # sl = lrelu(t + bzc); S = sum(sl)  (single fused Act op, fp32 in
            # -> bf16 out so the slb2 broadcast copy is a bf16->bf16 pattern)
            mS = sp.tile([PT, LD + 1], FP32, tag="mS")
            sl = sp.tile([PT, L], BF16, tag="sl")
            nc.scalar.activation(
                out=sl[:], in_=t16[:], func=AF.Lrelu,
                bias=bzc[:], alpha=0.01, accum_out=mS[:, LD:LD + 1])

            # slb2: sl duplicated pairs in bf16 [P, L, 2]
            slb2 = sp.tile([PT, L, 2], BF16, tag="slb2")
            nc.vector.tensor_copy(
                out=slb2[:],
                in_=sl[:].rearrange("p (l o) -> p l o", o=1).to_broadcast([PT, L, 2]))

            # ---- m-pass: ym = ymv * sl, tree-fold over l ----
            if INPLACE_YM:
                ymt = ymv
            else:
                ymt = ymbp.tile([PT, L, LD], BF16, tag="ymb")
            ymi = ymv[:].rearrange("p l (o t) -> p l o t", o=LD // 2)
            ymo = ymt[:].rearrange("p l (o t) -> p l o t", o=LD // 2)
            nc.vector.tensor_mul(
                out=ymo, in0=ymi,
                in1=slb2[:].rearrange("p l (o t) -> p l o t", o=1)
                    .to_broadcast([PT, L, LD // 2, 2]))
            cur = ymt[:]
            n = L
            lvl = 0
            while n > 3:
                h = n // 2
                odd = n - 2 * h
                eng = nc.vector if lvl < DVE_TREE_LEVELS else gp_eng
                eng.tensor_add(out=cur[:, 0:h, :], in0=cur[:, 0:h, :],
                               in1=cur[:, h:2 * h, :])
                if odd:
                    eng.tensor_add(out=cur[:, 0:1, :], in0=cur[:, 0:1, :],
                                   in1=cur[:, 2 * h:n, :])
                n = h
                lvl += 1
            # final level(s) -> fp32 m' into mS[:, 0:64]
            m1 = mS[:, 0:LD].rearrange("p (o d) -> p o d", o=1)
            gp_eng.tensor_add(out=m1, in0=cur[:, 0:1, :], in1=cur[:, 1:2, :])
            if n == 3:
                gp_eng.tensor_add(out=m1, in0=m1, in1=cur[:, 2:3, :])

            # ---- normalization ----
            total = sp.tile([PT, 1], FP32, tag="total")
            nc.vector.tensor_add(out=total[:], in0=s0[:], in1=mS[:, LD:LD + 1])
            rc = sp.tile([PT, 1], FP32, tag="rc")
            nc.vector.reciprocal(out=rc[:], in_=total[:])

            # g_aug = [gt, 1] * n0s ; mSs = mS * rinv * rc
            g_aug = sp.tile([PT, GD + 1], FP32, tag="g_aug")
            gp_eng.tensor_mul(out=g_aug[:, GD:GD + 1], in0=s0[:], in1=rc[:])
            gp_eng.tensor_mul(
                out=g_aug[:, 0:GD], in0=gt[:],
                in1=g_aug[:, GD:GD + 1].to_broadcast([PT, GD]))
            mSs = sp.tile([PT, LD + 1], FP32, tag="mSs")
            gp_eng.tensor_mul(out=mSs[:], in0=mS[:], in1=rinv_sb[:])
            gp_eng.tensor_mul(out=mSs[:], in0=mSs[:],
                              in1=rc[:].to_broadcast([PT, LD + 1]))

            # ---- transposes + phase-A matmuls (bf16) ----
            pG = psA.tile([GD + 1, PT], FP32, tag="pG")
            nc.tensor.transpose(pG[:], g_aug[:], ident[:])
            gTs = fp.tile([GD + 1, PT], BF16, tag="gTs")
            nc.scalar.copy(out=gTs[:], in_=pG[:])

            pM = psA.tile([LD + 1, PT], FP32, tag="pG")
            nc.tensor.transpose(pM[:], mSs[:], ident[:])
            msts = fp.tile([LD + 1, PT], BF16, tag="msts")
            nc.scalar.copy(out=msts[:], in_=pM[:])

            sh = states_halves[it // 2]
            c0 = (it % 2) * PT
            pW = psB.tile([H, PT], FP32, tag="pW")
            nc.tensor.matmul(pW[:], lhsT=waug_sb[:], rhs=gTs[:], start=True, stop=True)
            nc.scalar.activation(out=sh[0:H, c0:c0 + PT], in_=pW[:], func=AF.Relu)
            pAg = psB.tile([H, PT], FP32, tag="pW")
            nc.tensor.matmul(pAg[:], lhsT=uaug_sb[:], rhs=msts[:], start=True, stop=True)
            nc.scalar.activation(out=sh[H:2 * H, c0:c0 + PT], in_=pAg[:], func=AF.Relu)

            emit_mlp(it)

    nc.compile()
    return nc


def _prep(inputs):
    import ml_dtypes
    W_w = np.asarray(inputs["W_w"], np.float32)
    W_b = np.asarray(inputs["W_b"], np.float32)
    U_w = np.asarray(inputs["U_w"], np.float32)
    U_b = np.asarray(inputs["U_b"], np.float32)
    att_w = np.asarray(inputs["att_w"], np.float32)
    att_b = np.asarray(inputs["att_b"], np.float32)
    l1_w = np.asarray(inputs["l1_w"], np.float32)
    l1_b = np.asarray(inputs["l1_b"], np.float32)
    l2_w = np.asarray(inputs["l2_w"], np.float32)
    l2_b = np.asarray(inputs["l2_b"], np.float32)
    l3_w = np.asarray(inputs["l3_w"], np.float32)
    l3_b = np.asarray(inputs["l3_b"], np.float32)

    a_g, a_l = att_w[0, :H], att_w[0, H:]
    v = U_w.T @ a_l
    p = W_w.T @ a_g
    q = W_w.T @ a_l
    c_g = float(a_g @ W_b)
    c_q = float(a_l @ W_b)
    c_v = float(a_l @ U_b)
    ab = float(att_b[0])
    c_z = c_g + c_v + ab
    c_s0 = c_g + c_q + ab

    bf = ml_dtypes.bfloat16
    # 1/v computed against the bf16-rounded v actually used on device
    v16 = v.astype(bf)
    v16f = v16.astype(np.float32)
    v16f = np.where(np.abs(v16f) < 1e-20, 1e-20, v16f)
    rinv_row = np.concatenate([1.0 / v16f, [1.0]]).astype(np.float32)

    consts = dict(
        waug=np.ascontiguousarray(np.vstack([W_w.T, W_b[None, :]]).astype(bf)),
        uaug=np.ascontiguousarray(np.vstack([U_w.T, U_b[None, :]]).astype(bf)),
        vb16=np.ascontiguousarray(np.broadcast_to(v16, (128, LD))),
        rinv=np.ascontiguousarray(np.broadcast_to(rinv_row, (128, LD + 1))),
        pb=np.ascontiguousarray(np.broadcast_to(p, (128, GD)).astype(np.float32)),
        pqb=np.ascontiguousarray(np.broadcast_to(p + q, (128, GD)).astype(np.float32)),
        l1wt=np.ascontiguousarray(l1_w.T.astype(bf)),
        l1b=np.ascontiguousarray(l1_b.reshape(2, 128).T.astype(np.float32)),
        l2wt=np.ascontiguousarray(l2_w.T.astype(bf)),
        l2b=np.ascontiguousarray(l2_b.reshape(2, 128).T.astype(np.float32)),
        l3wt=np.ascontiguousarray(l3_w.T.astype(bf)),
        l3b=np.ascontiguousarray(l3_b[:, None].astype(np.float32)),
    )
    return consts, c_z, c_s0


def _get_graph_and_consts(inputs):
    consts, c_z, c_s0 = _prep(inputs)
    key = (c_z, c_s0)
    if key not in _CACHE:
        _CACHE[key] = build_graph(c_z, c_s0)
    return _CACHE[key], consts


def kernel(**inputs) -> np.ndarray:
    nc, consts = _get_graph_and_consts(inputs)
    gs = np.ascontiguousarray(np.asarray(inputs["global_states"], np.float32))
    ls = np.ascontiguousarray(np.asarray(inputs["local_states"], np.float32))
    in_maps = []
    for i in range(NCORES):
        m = dict(consts)
        m["x"] = np.ascontiguousarray(ls[i * BC:(i + 1) * BC])
        m["g"] = np.ascontiguousarray(gs[i * BC:(i + 1) * BC])
        in_maps.append(m)
    res = run_bass_kernel_spmd(nc, in_maps, list(range(NCORES)))
    outs = [res.results[i]["out"] for i in range(NCORES)]
    return np.concatenate(outs, axis=0).astype(np.float32)


# revision 5
# speedup vs baseline: 1.1879x; 1.0020x over previous
"""Trainium2 Bass kernel for nn_Actor (GNN message-passing actor network), v2.

Math (per sample b):
  v  = U_w.T @ a_l ; p = W_w.T @ a_g ; q = W_w.T @ a_l
  c_z  = a_g.W_b + a_l.U_b + att_b ; c_s0 = a_g.W_b + a_l.W_b + att_b
  ymv[b,l,d] = x[b,l,d] * v[d]                (bf16)
  t[b,l]  = sum_d ymv                         (tensor_reduce over d)
  sl      = lrelu(t + G.p + c_z);  s0 = lrelu(G.(p+q) + c_s0)
  total   = s0 + sum sl;  S = sum sl
  m'[b,d] = sum_l sl*ymv = v[d] * sum_l sl*x  (bf16 mult + in-place tree)
  mSs     = [m' * (1/v), S] / total ; g_aug = [G, 1]*s0/total
  states  = relu([ [W_w.T; W_b]^T g_aug ; [U_w.T; U_b]^T mSs ])
  out     = sigmoid(l3(relu(l2(relu(l1(states))))))

Sharding: pure data parallel, batch 4096 -> 8 cores x 512.
Engine split: Act converts x->bf16 + all activation/bias ops; DVE does the two
big bf16 multiplies + reduce + tree top; GpSimd does tree tail + small scaling;
PE does transposes + all matmuls in bf16.
"""

import numpy as np
from contextlib import ExitStack

import concourse.bass as bass
import concourse.bacc as bacc
import concourse.tile as tile
from concourse import masks, mybir
from concourse.bass_utils import run_bass_kernel_spmd

FP32 = mybir.dt.float32
BF16 = mybir.dt.bfloat16
AX = mybir.AxisListType
OP = mybir.AluOpType
AF = mybir.ActivationFunctionType

B, L = 4096, 200
GD, LD, AD, H = 64, 64, 8, 32
NCORES = 8
BC = B // NCORES          # 512 samples per core
PT = 128                  # samples per tile
NT = BC // PT             # 4 tiles per core

# how many tree levels stay on DVE before handing to GpSimd (rest of levels)
DVE_TREE_LEVELS = 2
USE_TTR = False      # tensor_tensor_reduce for the G-dot products
GP_TAILS = False     # gpsimd handles tree tails + small scaling ops
INPLACE_YM = True    # m-pass multiply in place on ymv

_CACHE = {}


def build_graph(c_z: float, c_s0: float):
    nc = bacc.Bacc()
    x = nc.declare_dram_parameter("x", [BC, L, LD], FP32, isOutput=False)
    g = nc.declare_dram_parameter("g", [BC, GD], FP32, isOutput=False)
    waug = nc.declare_dram_parameter("waug", [GD + 1, H], BF16, isOutput=False)
    uaug = nc.declare_dram_parameter("uaug", [LD + 1, H], BF16, isOutput=False)
    vb16 = nc.declare_dram_parameter("vb16", [128, LD], BF16, isOutput=False)
    rinv = nc.declare_dram_parameter("rinv", [128, LD + 1], FP32, isOutput=False)
    pb = nc.declare_dram_parameter("pb", [128, GD], FP32, isOutput=False)
    pqb = nc.declare_dram_parameter("pqb", [128, GD], FP32, isOutput=False)
    l1wt = nc.declare_dram_parameter("l1wt", [GD, 256], BF16, isOutput=False)
    l1b = nc.declare_dram_parameter("l1b", [128, 2], FP32, isOutput=False)
    l2wt = nc.declare_dram_parameter("l2wt", [256, 256], BF16, isOutput=False)
    l2b = nc.declare_dram_parameter("l2b", [128, 2], FP32, isOutput=False)
    l3wt = nc.declare_dram_parameter("l3wt", [256, AD], BF16, isOutput=False)
    l3b = nc.declare_dram_parameter("l3b", [AD, 1], FP32, isOutput=False)
    out = nc.declare_dram_parameter("out", [BC, AD], FP32, isOutput=True)

    with tile.TileContext(nc) as tc, ExitStack() as ctx:
        consts = ctx.enter_context(tc.tile_pool(name="consts", bufs=1))

        ident = consts.tile([128, 128], FP32)
        masks.make_identity(nc, ident[:])
        czt = consts.tile([128, 1], FP32)
        nc.vector.memset(czt[:], float(c_z))
        cst = consts.tile([128, 1], FP32)
        nc.vector.memset(cst[:], float(c_s0))

        waug_sb = consts.tile([GD + 1, H], BF16)
        nc.scalar.dma_start(out=waug_sb[:], in_=waug[:])
        uaug_sb = consts.tile([LD + 1, H], BF16)
        nc.scalar.dma_start(out=uaug_sb[:], in_=uaug[:])
        vb_sb = consts.tile([128, 1, LD], BF16)
        nc.sync.dma_start(out=vb_sb[:], in_=vb16[:].rearrange("p (o d) -> p o d", o=1))
        rinv_sb = consts.tile([128, LD + 1], FP32)
        nc.scalar.dma_start(out=rinv_sb[:], in_=rinv[:])
        pb_sb = consts.tile([128, GD], FP32)
        nc.scalar.dma_start(out=pb_sb[:], in_=pb[:])
        pqb_sb = consts.tile([128, GD], FP32)
        nc.scalar.dma_start(out=pqb_sb[:], in_=pqb[:])
        l1wt_sb = consts.tile([GD, 256], BF16)
        l1b_sb = consts.tile([128, 2], FP32)
        l2wt_a = consts.tile([128, 256], BF16)
        l2wt_b = consts.tile([128, 256], BF16)
        l2b_sb = consts.tile([128, 2], FP32)
        l3wt_a = consts.tile([128, AD], BF16)
        l3wt_b = consts.tile([128, AD], BF16)
        l3b_sb = consts.tile([AD, 1], FP32)

        def emit_mlp_weight_loads():
            # deferred so the Act sequencer isn't stuck issuing DMAs while
            # tile-0 conversions could run; first consumer is the MLP (~40us)
            nc.scalar.dma_start(out=l1wt_sb[:], in_=l1wt[:])
            nc.scalar.dma_start(out=l1b_sb[:], in_=l1b[:])
            nc.scalar.dma_start(out=l2wt_a[:], in_=l2wt[0:128])
            nc.scalar.dma_start(out=l2wt_b[:], in_=l2wt[128:256])
            nc.scalar.dma_start(out=l2b_sb[:], in_=l2b[:])
            nc.scalar.dma_start(out=l3wt_a[:], in_=l3wt[0:128])
            nc.scalar.dma_start(out=l3wt_b[:], in_=l3wt[128:256])
            nc.scalar.dma_start(out=l3b_sb[:], in_=l3b[:])

        NCH = 4                    # x DMA/convert chunks per tile
        LCH = L // NCH
        xfp = ctx.enter_context(tc.tile_pool(name="xfp", bufs=6))
        xbp = ctx.enter_context(tc.tile_pool(name="xbp", bufs=4))
        ymp = ctx.enter_context(tc.tile_pool(name="ymp", bufs=2))
        ymbp = ctx.enter_context(tc.tile_pool(name="ymbp", bufs=1))
        tdp = ctx.enter_context(tc.tile_pool(name="tdp", bufs=1))
        sp = ctx.enter_context(tc.tile_pool(name="sp", bufs=2))
        fp = ctx.enter_context(tc.tile_pool(name="fp", bufs=2))
        glob = ctx.enter_context(tc.tile_pool(name="glob", bufs=1))
        psA = ctx.enter_context(tc.tile_pool(name="psA", bufs=2, space="PSUM"))
        psB = ctx.enter_context(tc.tile_pool(name="psB", bufs=2, space="PSUM"))
        ps1 = ctx.enter_context(tc.tile_pool(name="ps1", bufs=1, space="PSUM"))

        st_h0 = glob.tile([2 * H, BC // 2], BF16, tag="st_h0")
        st_h1 = glob.tile([2 * H, BC // 2], BF16, tag="st_h1")
        states_halves = [st_h0, st_h1]

        BH = PT

        def emit_mlp(it):
            """MLP for one tile's 128 samples; emitted right after the tile's
            states columns complete so it overlaps later tiles' streaming."""
            str_ = states_halves[it // 2][:, (it % 2) * PT:(it % 2) * PT + PT]

            p1a = ps1.tile([128, BH], FP32, tag="p1a")
            nc.tensor.matmul(p1a[:], lhsT=l1wt_sb[:, 0:128], rhs=str_, start=True, stop=True)
            p1b = ps1.tile([128, BH], FP32, tag="p1b")
            nc.tensor.matmul(p1b[:], lhsT=l1wt_sb[:, 128:256], rhs=str_, start=True, stop=True)
            a1a = fp.tile([128, BH], BF16, tag="a1a")
            nc.scalar.activation(out=a1a[:], in_=p1a[:], func=AF.Relu,
                                 bias=l1b_sb[:, 0:1], scale=1.0)
            a1b = fp.tile([128, BH], BF16, tag="a1b")
            nc.scalar.activation(out=a1b[:], in_=p1b[:], func=AF.Relu,
                                 bias=l1b_sb[:, 1:2], scale=1.0)

            p2a = ps1.tile([128, BH], FP32, tag="p1a")
            nc.tensor.matmul(p2a[:], lhsT=l2wt_a[:, 0:128], rhs=a1a[:], start=True, stop=False)
            nc.tensor.matmul(p2a[:], lhsT=l2wt_b[:, 0:128], rhs=a1b[:], start=False, stop=True)
            p2b = ps1.tile([128, BH], FP32, tag="p1b")
            nc.tensor.matmul(p2b[:], lhsT=l2wt_a[:, 128:256], rhs=a1a[:], start=True, stop=False)
            nc.tensor.matmul(p2b[:], lhsT=l2wt_b[:, 128:256], rhs=a1b[:], start=False, stop=True)
            a2a = fp.tile([128, BH], BF16, tag="a2a")
            nc.scalar.activation(out=a2a[:], in_=p2a[:], func=AF.Relu,
                                 bias=l2b_sb[:, 0:1], scale=1.0)
            a2b = fp.tile([128, BH], BF16, tag="a2b")
            nc.scalar.activation(out=a2b[:], in_=p2b[:], func=AF.Relu,
                                 bias=l2b_sb[:, 1:2], scale=1.0)

            p3 = ps1.tile([AD, BH], FP32, tag="p1a")
            nc.tensor.matmul(p3[:], lhsT=l3wt_a[:], rhs=a2a[:], start=True, stop=False)
            nc.tensor.matmul(p3[:], lhsT=l3wt_b[:], rhs=a2b[:], start=False, stop=True)
            oT = fp.tile([AD, BH], FP32, tag="oT")
            nc.scalar.activation(out=oT[:], in_=p3[:], func=AF.Sigmoid,
                                 bias=l3b_sb[:], scale=1.0)

            b0o = it * PT
            pO = psA.tile([PT, AD], FP32, tag="pO")
            nc.tensor.transpose(pO[:], oT[:], ident[0:AD, 0:AD])
            ob = fp.tile([PT, AD], FP32, tag="ob")
            nc.scalar.copy(out=ob[:], in_=pO[:])
            nc.sync.dma_start(out=out[b0o:b0o + PT, :], in_=ob[:])

        def emit_load(it):
            """DMA x in chunks; convert fp32->bf16 on Act; multiply by v on
            DVE per chunk. Returns (ymv, gt) for the tile."""
            b0 = it * PT
            ymv = ymp.tile([PT, L, LD], BF16, tag="ymv")
            for cchunk in range(NCH):
                ls = slice(cchunk * LCH, (cchunk + 1) * LCH)
                xf = xfp.tile([PT, LCH, LD], FP32, tag="xf")
                nc.sync.dma_start(out=xf[:], in_=x[b0:b0 + PT, ls])
                xbh = xbp.tile([PT, LCH, LD], BF16, tag="xbh")
                nc.scalar.copy(out=xbh[:], in_=xf[:])
                nc.vector.tensor_mul(
                    out=ymv[:, ls, :], in0=xbh[:],
                    in1=vb_sb[:].to_broadcast([PT, LCH, LD]))
            gt = sp.tile([PT, GD], FP32, tag="gt")
            nc.sync.dma_start(out=gt[:], in_=g[b0:b0 + PT])
            return ymv, gt

        loaded = emit_load(0)
        for it in range(NT):
            b0 = it * PT
            ymv, gt = loaded

            # ---- per-sample bias dots ----
            junk = sp.tile([PT, GD], FP32, tag="junk")
            bzc = sp.tile([PT, 1], FP32, tag="bzc")
            s0z = sp.tile([PT, 1], FP32, tag="s0z")
            if USE_TTR:
                nc.vector.tensor_tensor_reduce(
                    out=junk[:], in0=gt[:], in1=pb_sb[:], scale=1.0, scalar=float(c_z),
                    op0=OP.mult, op1=OP.add, accum_out=bzc[:])
                nc.vector.tensor_tensor_reduce(
                    out=junk[:], in0=gt[:], in1=pqb_sb[:], scale=1.0, scalar=float(c_s0),
                    op0=OP.mult, op1=OP.add, accum_out=s0z[:])
            else:
                bz0 = sp.tile([PT, 1], FP32, tag="bz0")
                nc.vector.tensor_mul(out=junk[:], in0=gt[:], in1=pb_sb[:])
                nc.vector.reduce_sum(out=bz0[:], in_=junk[:], axis=AX.X)
                nc.scalar.activation(out=bzc[:], in_=bz0[:], func=AF.Identity,
                                     bias=czt[:, 0:1])
                nc.vector.tensor_mul(out=junk[:], in0=gt[:], in1=pqb_sb[:])
                nc.vector.reduce_sum(out=bz0[:], in_=junk[:], axis=AX.X)
                nc.scalar.activation(out=s0z[:], in_=bz0[:], func=AF.Identity,
                                     bias=cst[:, 0:1])
            s0 = sp.tile([PT, 1], FP32, tag="s0")
            nc.scalar.activation(out=s0[:], in_=s0z[:], func=AF.Lrelu, alpha=0.01)

            # ---- t-pass: tree-fold ymv over d into t16 ----
            td = tdp.tile([PT, L, LD // 2], BF16, tag="td")
            if it == 0:
                # split L1 per chunk: overlaps the initial conversions
                for cchunk in range(NCH):
                    ls = slice(cchunk * LCH, (cchunk + 1) * LCH)
                    nc.vector.tensor_add(out=td[:, ls, :],
                                         in0=ymv[:, ls, 0:LD // 2],
                                         in1=ymv[:, ls, LD // 2:LD])
            else:
                nc.vector.tensor_add(out=td[:], in0=ymv[:, :, 0:LD // 2],
                                     in1=ymv[:, :, LD // 2:LD])
            gp_eng = nc.gpsimd if GP_TAILS else nc.vector
            dh = LD // 2
            lvl = 1
            while dh > 2:
                h = dh // 2
                eng = nc.vector if lvl < DVE_TREE_LEVELS else gp_eng
                eng.tensor_add(out=td[:, :, 0:h], in0=td[:, :, 0:h],
                               in1=td[:, :, h:dh])
                dh = h
                lvl += 1
            t16 = sp.tile([PT, L], FP32, tag="t16")
            gp_eng.tensor_add(
                out=t16[:].rearrange("p (l o) -> p l o", o=1),
                in0=td[:, :, 0:1], in1=td[:, :, 1:2])

            # ---- software pipelining: enqueue next tile's load NOW, so the
            # Act queue does conversions (and DVE the v-mults) while this
            # tile's t16 -> sl -> slb2 cross-engine round-trip is in flight.
            if it + 1 < NT:
                loaded = emit_load(it + 1)

            # sl = lrelu(t + bzc); S = sum(sl)  (single fused Act op, fp32 in
            # -> bf16 out so the slb2 broadcast copy is a bf16->bf16 pattern)
            mS = sp.tile([PT, LD + 1], FP32, tag="mS")
            sl = sp.tile([PT, L], BF16, tag="sl")
            nc.scalar.activation(
                out=sl[:], in_=t16[:], func=AF.Lrelu,
                bias=bzc[:], alpha=0.01, accum_out=mS[:, LD:LD + 1])

            if it == 0:
                emit_mlp_weight_loads()

            # slb2: sl duplicated pairs in bf16 [P, L, 2]
            slb2 = sp.tile([PT, L, 2], BF16, tag="slb2")
            nc.vector.tensor_copy(
                out=slb2[:],
                in_=sl[:].rearrange("p (l o) -> p l o", o=1).to_broadcast([PT, L, 2]))

            # ---- m-pass: ym = ymv * sl, tree-fold over l ----
            if INPLACE_YM:
                ymt = ymv
            else:
                ymt = ymbp.tile([PT, L, LD], BF16, tag="ymb")
            ymi = ymv[:].rearrange("p l (o t) -> p l o t", o=LD // 2)
            ymo = ymt[:].rearrange("p l (o t) -> p l o t", o=LD // 2)
            nc.vector.tensor_mul(
                out=ymo, in0=ymi,
                in1=slb2[:].rearrange("p l (o t) -> p l o t", o=1)
                    .to_broadcast([PT, L, LD // 2, 2]))
            cur = ymt[:]
            n = L
            lvl = 0
            while n > 3:
                h = n // 2
                odd = n - 2 * h
                eng = nc.vector if lvl < DVE_TREE_LEVELS else gp_eng
                eng.tensor_add(out=cur[:, 0:h, :], in0=cur[:, 0:h, :],
                               in1=cur[:, h:2 * h, :])
                if odd:
                    eng.tensor_add(out=cur[:, 0:1, :], in0=cur[:, 0:1, :],
                                   in1=cur[:, 2 * h:n, :])
                n = h
                lvl += 1
            # final level(s) -> fp32 m' into mS[:, 0:64]
            m1 = mS[:, 0:LD].rearrange("p (o d) -> p o d", o=1)
            gp_eng.tensor_add(out=m1, in0=cur[:, 0:1, :], in1=cur[:, 1:2, :])
            if n == 3:
                gp_eng.tensor_add(out=m1, in0=m1, in1=cur[:, 2:3, :])

            # ---- normalization ----
            total = sp.tile([PT, 1], FP32, tag="total")
            nc.vector.tensor_add(out=total[:], in0=s0[:], in1=mS[:, LD:LD + 1])
            rc = sp.tile([PT, 1], FP32, tag="rc")
            nc.vector.reciprocal(out=rc[:], in_=total[:])

            # g_aug = [gt, 1] * n0s ; mSs = mS * rinv * rc
            g_aug = sp.tile([PT, GD + 1], FP32, tag="g_aug")
            gp_eng.tensor_mul(out=g_aug[:, GD:GD + 1], in0=s0[:], in1=rc[:])
            gp_eng.tensor_mul(
                out=g_aug[:, 0:GD], in0=gt[:],
                in1=g_aug[:, GD:GD + 1].to_broadcast([PT, GD]))
            mSs = sp.tile([PT, LD + 1], FP32, tag="mSs")
            gp_eng.tensor_mul(out=mSs[:], in0=mS[:], in1=rinv_sb[:])
            gp_eng.tensor_mul(out=mSs[:], in0=mSs[:],
                              in1=rc[:].to_broadcast([PT, LD + 1]))

            # ---- transposes + phase-A matmuls (bf16) ----
            pG = psA.tile([GD + 1, PT], FP32, tag="pG")
            nc.tensor.transpose(pG[:], g_aug[:], ident[:])
            gTs = fp.tile([GD + 1, PT], BF16, tag="gTs")
            nc.scalar.copy(out=gTs[:], in_=pG[:])

            pM = psA.tile([LD + 1, PT], FP32, tag="pG")
            nc.tensor.transpose(pM[:], mSs[:], ident[:])
            msts = fp.tile([LD + 1, PT], BF16, tag="msts")
            nc.scalar.copy(out=msts[:], in_=pM[:])

            sh = states_halves[it // 2]
            c0 = (it % 2) * PT
            pW = psB.tile([H, PT], FP32, tag="pW")
            nc.tensor.matmul(pW[:], lhsT=waug_sb[:], rhs=gTs[:], start=True, stop=True)
            nc.scalar.activation(out=sh[0:H, c0:c0 + PT], in_=pW[:], func=AF.Relu)
            pAg = psB.tile([H, PT], FP32, tag="pW")
            nc.tensor.matmul(pAg[:], lhsT=uaug_sb[:], rhs=msts[:], start=True, stop=True)
            nc.scalar.activation(out=sh[H:2 * H, c0:c0 + PT], in_=pAg[:], func=AF.Relu)

            emit_mlp(it)

    nc.compile()
    return nc


def _prep(inputs):
    import ml_dtypes
    W_w = np.asarray(inputs["W_w"], np.float32)
    W_b = np.asarray(inputs["W_b"], np.float32)
    U_w = np.asarray(inputs["U_w"], np.float32)
    U_b = np.asarray(inputs["U_b"], np.float32)
    att_w = np.asarray(inputs["att_w"], np.float32)
    att_b = np.asarray(inputs["att_b"], np.float32)
    l1_w = np.asarray(inputs["l1_w"], np.float32)
    l1_b = np.asarray(inputs["l1_b"], np.float32)
    l2_w = np.asarray(inputs["l2_w"], np.float32)
    l2_b = np.asarray(inputs["l2_b"], np.float32)
    l3_w = np.asarray(inputs["l3_w"], np.float32)
    l3_b = np.asarray(inputs["l3_b"], np.float32)

    a_g, a_l = att_w[0, :H], att_w[0, H:]
    v = U_w.T @ a_l
    p = W_w.T @ a_g
    q = W_w.T @ a_l
    c_g = float(a_g @ W_b)
    c_q = float(a_l @ W_b)
    c_v = float(a_l @ U_b)
    ab = float(att_b[0])
    c_z = c_g + c_v + ab
    c_s0 = c_g + c_q + ab

    bf = ml_dtypes.bfloat16
    # 1/v computed against the bf16-rounded v actually used on device
    v16 = v.astype(bf)
    v16f = v16.astype(np.float32)
    v16f = np.where(np.abs(v16f) < 1e-20, 1e-20, v16f)
    rinv_row = np.concatenate([1.0 / v16f, [1.0]]).astype(np.float32)

    consts = dict(
        waug=np.ascontiguousarray(np.vstack([W_w.T, W_b[None, :]]).astype(bf)),
        uaug=np.ascontiguousarray(np.vstack([U_w.T, U_b[None, :]]).astype(bf)),
        vb16=np.ascontiguousarray(np.broadcast_to(v16, (128, LD))),
        rinv=np.ascontiguousarray(np.broadcast_to(rinv_row, (128, LD + 1))),
        pb=np.ascontiguousarray(np.broadcast_to(p, (128, GD)).astype(np.float32)),
        pqb=np.ascontiguousarray(np.broadcast_to(p + q, (128, GD)).astype(np.float32)),
        l1wt=np.ascontiguousarray(l1_w.T.astype(bf)),
        l1b=np.ascontiguousarray(l1_b.reshape(2, 128).T.astype(np.float32)),
        l2wt=np.ascontiguousarray(l2_w.T.astype(bf)),
        l2b=np.ascontiguousarray(l2_b.reshape(2, 128).T.astype(np.float32)),
        l3wt=np.ascontiguousarray(l3_w.T.astype(bf)),
        l3b=np.ascontiguousarray(l3_b[:, None].astype(np.float32)),
    )
    return consts, c_z, c_s0


def _get_graph_and_consts(inputs):
    consts, c_z, c_s0 = _prep(inputs)
    key = (c_z, c_s0)
    if key not in _CACHE:
        _CACHE[key] = build_graph(c_z, c_s0)
    return _CACHE[key], consts


def kernel(**inputs) -> np.ndarray:
    nc, consts = _get_graph_and_consts(inputs)
    gs = np.ascontiguousarray(np.asarray(inputs["global_states"], np.float32))
    ls = np.ascontiguousarray(np.asarray(inputs["local_states"], np.float32))
    in_maps = []
    for i in range(NCORES):
        m = dict(consts)
        m["x"] = np.ascontiguousarray(ls[i * BC:(i + 1) * BC])
        m["g"] = np.ascontiguousarray(gs[i * BC:(i + 1) * BC])
        in_maps.append(m)
    res = run_bass_kernel_spmd(nc, in_maps, list(range(NCORES)))
    outs = [res.results[i]["out"] for i in range(NCORES)]
    return np.concatenate(outs, axis=0).astype(np.float32)


# revision 6
# speedup vs baseline: 1.1894x; 1.0012x over previous
"""Trainium2 Bass kernel for nn_Actor (GNN message-passing actor network), v2.

Math (per sample b):
  v  = U_w.T @ a_l ; p = W_w.T @ a_g ; q = W_w.T @ a_l
  c_z  = a_g.W_b + a_l.U_b + att_b ; c_s0 = a_g.W_b + a_l.W_b + att_b
  ymv[b,l,d] = x[b,l,d] * v[d]                (bf16)
  t[b,l]  = sum_d ymv                         (tensor_reduce over d)
  sl      = lrelu(t + G.p + c_z);  s0 = lrelu(G.(p+q) + c_s0)
  total   = s0 + sum sl;  S = sum sl
  m'[b,d] = sum_l sl*ymv = v[d] * sum_l sl*x  (bf16 mult + in-place tree)
  mSs     = [m' * (1/v), S] / total ; g_aug = [G, 1]*s0/total
  states  = relu([ [W_w.T; W_b]^T g_aug ; [U_w.T; U_b]^T mSs ])
  out     = sigmoid(l3(relu(l2(relu(l1(states))))))

Sharding: pure data parallel, batch 4096 -> 8 cores x 512.
Engine split: Act converts x->bf16 + all activation/bias ops; DVE does the two
big bf16 multiplies + reduce + tree top; GpSimd does tree tail + small scaling;
PE does transposes + all matmuls in bf16.
"""

import numpy as np
from contextlib import ExitStack

import concourse.bass as bass
import concourse.bacc as bacc
import concourse.tile as tile
from concourse import masks, mybir
from concourse.bass_utils import run_bass_kernel_spmd

FP32 = mybir.dt.float32
BF16 = mybir.dt.bfloat16
AX = mybir.AxisListType
OP = mybir.AluOpType
AF = mybir.ActivationFunctionType

B, L = 4096, 200
GD, LD, AD, H = 64, 64, 8, 32
NCORES = 8
BC = B // NCORES          # 512 samples per core
PT = 128                  # samples per tile
NT = BC // PT             # 4 tiles per core

# how many tree levels stay on DVE before handing to GpSimd (rest of levels)
DVE_TREE_LEVELS = 2
USE_TTR = False      # tensor_tensor_reduce for the G-dot products
GP_TAILS = False     # gpsimd handles tree tails + small scaling ops
INPLACE_YM = True    # m-pass multiply in place on ymv

_CACHE = {}


def build_graph(c_z: float, c_s0: float):
    nc = bacc.Bacc()
    x = nc.declare_dram_parameter("x", [BC, L, LD], FP32, isOutput=False)
    g = nc.declare_dram_parameter("g", [BC, GD], FP32, isOutput=False)
    waug = nc.declare_dram_parameter("waug", [GD + 1, H], BF16, isOutput=False)
    uaug = nc.declare_dram_parameter("uaug", [LD + 1, H], BF16, isOutput=False)
    vb16 = nc.declare_dram_parameter("vb16", [128, LD], BF16, isOutput=False)
    rinv = nc.declare_dram_parameter("rinv", [128, LD + 1], FP32, isOutput=False)
    pb = nc.declare_dram_parameter("pb", [128, GD], FP32, isOutput=False)
    pqb = nc.declare_dram_parameter("pqb", [128, GD], FP32, isOutput=False)
    l1wt = nc.declare_dram_parameter("l1wt", [GD, 256], BF16, isOutput=False)
    l1b = nc.declare_dram_parameter("l1b", [128, 2], FP32, isOutput=False)
    l2wt = nc.declare_dram_parameter("l2wt", [256, 256], BF16, isOutput=False)
    l2b = nc.declare_dram_parameter("l2b", [128, 2], FP32, isOutput=False)
    l3wt = nc.declare_dram_parameter("l3wt", [256, AD], BF16, isOutput=False)
    l3b = nc.declare_dram_parameter("l3b", [AD, 1], FP32, isOutput=False)
    out = nc.declare_dram_parameter("out", [BC, AD], FP32, isOutput=True)

    with tile.TileContext(nc) as tc, ExitStack() as ctx:
        consts = ctx.enter_context(tc.tile_pool(name="consts", bufs=1))

        ident = consts.tile([128, 128], FP32)
        masks.make_identity(nc, ident[:])
        czt = consts.tile([128, 1], FP32)
        nc.vector.memset(czt[:], float(c_z))
        cst = consts.tile([128, 1], FP32)
        nc.vector.memset(cst[:], float(c_s0))

        waug_sb = consts.tile([GD + 1, H], BF16)
        nc.scalar.dma_start(out=waug_sb[:], in_=waug[:])
        uaug_sb = consts.tile([LD + 1, H], BF16)
        nc.scalar.dma_start(out=uaug_sb[:], in_=uaug[:])
        vb_sb = consts.tile([128, 1, LD], BF16)
        nc.sync.dma_start(out=vb_sb[:], in_=vb16[:].rearrange("p (o d) -> p o d", o=1))
        rinv_sb = consts.tile([128, LD + 1], FP32)
        nc.scalar.dma_start(out=rinv_sb[:], in_=rinv[:])
        pb_sb = consts.tile([128, GD], FP32)
        nc.scalar.dma_start(out=pb_sb[:], in_=pb[:])
        pqb_sb = consts.tile([128, GD], FP32)
        nc.scalar.dma_start(out=pqb_sb[:], in_=pqb[:])
        l1wt_sb = consts.tile([GD, 256], BF16)
        l1b_sb = consts.tile([128, 2], FP32)
        l2wt_a = consts.tile([128, 256], BF16)
        l2wt_b = consts.tile([128, 256], BF16)
        l2b_sb = consts.tile([128, 2], FP32)
        l3wt_a = consts.tile([128, AD], BF16)
        l3wt_b = consts.tile([128, AD], BF16)
        l3b_sb = consts.tile([AD, 1], FP32)

        def emit_mlp_weight_loads():
            # deferred so the Act sequencer isn't stuck issuing DMAs while
            # tile-0 conversions could run; first consumer is the MLP (~40us)
            nc.scalar.dma_start(out=l1wt_sb[:], in_=l1wt[:])
            nc.scalar.dma_start(out=l1b_sb[:], in_=l1b[:])
            nc.scalar.dma_start(out=l2wt_a[:], in_=l2wt[0:128])
            nc.scalar.dma_start(out=l2wt_b[:], in_=l2wt[128:256])
            nc.scalar.dma_start(out=l2b_sb[:], in_=l2b[:])
            nc.scalar.dma_start(out=l3wt_a[:], in_=l3wt[0:128])
            nc.scalar.dma_start(out=l3wt_b[:], in_=l3wt[128:256])
            nc.scalar.dma_start(out=l3b_sb[:], in_=l3b[:])

        NCH = 4                    # x DMA/convert chunks per tile
        LCH = L // NCH
        xfp = ctx.enter_context(tc.tile_pool(name="xfp", bufs=4))
        xf0p = ctx.enter_context(tc.tile_pool(name="xf0p", bufs=4))
        xb0p = ctx.enter_context(tc.tile_pool(name="xb0p", bufs=4))
        xbp = ctx.enter_context(tc.tile_pool(name="xbp", bufs=4))
        ymp = ctx.enter_context(tc.tile_pool(name="ymp", bufs=2))
        ymbp = ctx.enter_context(tc.tile_pool(name="ymbp", bufs=1))
        tdp = ctx.enter_context(tc.tile_pool(name="tdp", bufs=1))
        sp = ctx.enter_context(tc.tile_pool(name="sp", bufs=2))
        fp = ctx.enter_context(tc.tile_pool(name="fp", bufs=2))
        glob = ctx.enter_context(tc.tile_pool(name="glob", bufs=1))
        psA = ctx.enter_context(tc.tile_pool(name="psA", bufs=2, space="PSUM"))
        psB = ctx.enter_context(tc.tile_pool(name="psB", bufs=2, space="PSUM"))
        ps1 = ctx.enter_context(tc.tile_pool(name="ps1", bufs=1, space="PSUM"))

        st_h0 = glob.tile([2 * H, BC // 2], BF16, tag="st_h0")
        st_h1 = glob.tile([2 * H, BC // 2], BF16, tag="st_h1")
        states_halves = [st_h0, st_h1]

        BH = PT

        def emit_mlp(it):
            """MLP for one tile's 128 samples; emitted right after the tile's
            states columns complete so it overlaps later tiles' streaming."""
            str_ = states_halves[it // 2][:, (it % 2) * PT:(it % 2) * PT + PT]

            p1a = ps1.tile([128, BH], FP32, tag="p1a")
            nc.tensor.matmul(p1a[:], lhsT=l1wt_sb[:, 0:128], rhs=str_, start=True, stop=True)
            p1b = ps1.tile([128, BH], FP32, tag="p1b")
            nc.tensor.matmul(p1b[:], lhsT=l1wt_sb[:, 128:256], rhs=str_, start=True, stop=True)
            a1a = fp.tile([128, BH], BF16, tag="a1a")
            nc.scalar.activation(out=a1a[:], in_=p1a[:], func=AF.Relu,
                                 bias=l1b_sb[:, 0:1], scale=1.0)
            a1b = fp.tile([128, BH], BF16, tag="a1b")
            nc.scalar.activation(out=a1b[:], in_=p1b[:], func=AF.Relu,
                                 bias=l1b_sb[:, 1:2], scale=1.0)

            p2a = ps1.tile([128, BH], FP32, tag="p1a")
            nc.tensor.matmul(p2a[:], lhsT=l2wt_a[:, 0:128], rhs=a1a[:], start=True, stop=False)
            nc.tensor.matmul(p2a[:], lhsT=l2wt_b[:, 0:128], rhs=a1b[:], start=False, stop=True)
            p2b = ps1.tile([128, BH], FP32, tag="p1b")
            nc.tensor.matmul(p2b[:], lhsT=l2wt_a[:, 128:256], rhs=a1a[:], start=True, stop=False)
            nc.tensor.matmul(p2b[:], lhsT=l2wt_b[:, 128:256], rhs=a1b[:], start=False, stop=True)
            a2a = fp.tile([128, BH], BF16, tag="a2a")
            nc.scalar.activation(out=a2a[:], in_=p2a[:], func=AF.Relu,
                                 bias=l2b_sb[:, 0:1], scale=1.0)
            a2b = fp.tile([128, BH], BF16, tag="a2b")
            nc.scalar.activation(out=a2b[:], in_=p2b[:], func=AF.Relu,
                                 bias=l2b_sb[:, 1:2], scale=1.0)

            p3 = ps1.tile([AD, BH], FP32, tag="p1a")
            nc.tensor.matmul(p3[:], lhsT=l3wt_a[:], rhs=a2a[:], start=True, stop=False)
            nc.tensor.matmul(p3[:], lhsT=l3wt_b[:], rhs=a2b[:], start=False, stop=True)
            oT = fp.tile([AD, BH], FP32, tag="oT")
            nc.scalar.activation(out=oT[:], in_=p3[:], func=AF.Sigmoid,
                                 bias=l3b_sb[:], scale=1.0)

            b0o = it * PT
            pO = psA.tile([PT, AD], FP32, tag="pO")
            nc.tensor.transpose(pO[:], oT[:], ident[0:AD, 0:AD])
            ob = fp.tile([PT, AD], FP32, tag="ob")
            nc.scalar.copy(out=ob[:], in_=pO[:])
            nc.sync.dma_start(out=out[b0o:b0o + PT, :], in_=ob[:])

        def emit_load(it):
            """DMA x in chunks; convert fp32->bf16 on Act; multiply by v on
            DVE per chunk. Tile 0 uses 8 small chunks so the first
            conversion starts before the DMA clock has ramped up.
            Returns (ymv, gt) for the tile."""
            b0 = it * PT
            ymv = ymp.tile([PT, L, LD], BF16, tag="ymv")
            nch = 8 if it == 0 else NCH
            lch = L // nch
            for cchunk in range(nch):
                ls = slice(cchunk * lch, (cchunk + 1) * lch)
                if it == 0:
                    xf = xf0p.tile([PT, lch, LD], FP32, tag="xf0")
                    xbh = xb0p.tile([PT, lch, LD], BF16, tag="xbh0")
                else:
                    xf = xfp.tile([PT, lch, LD], FP32, tag="xf")
                    xbh = xbp.tile([PT, lch, LD], BF16, tag="xbh")
                nc.sync.dma_start(out=xf[:], in_=x[b0:b0 + PT, ls])
                nc.scalar.copy(out=xbh[:], in_=xf[:])
                nc.vector.tensor_mul(
                    out=ymv[:, ls, :], in0=xbh[:],
                    in1=vb_sb[:].to_broadcast([PT, lch, LD]))
            gt = sp.tile([PT, GD], FP32, tag="gt")
            nc.sync.dma_start(out=gt[:], in_=g[b0:b0 + PT])
            return ymv, gt

        loaded = emit_load(0)
        for it in range(NT):
            b0 = it * PT
            ymv, gt = loaded

            # ---- per-sample bias dots ----
            junk = sp.tile([PT, GD], FP32, tag="junk")
            bzc = sp.tile([PT, 1], FP32, tag="bzc")
            s0z = sp.tile([PT, 1], FP32, tag="s0z")
            if USE_TTR:
                nc.vector.tensor_tensor_reduce(
                    out=junk[:], in0=gt[:], in1=pb_sb[:], scale=1.0, scalar=float(c_z),
                    op0=OP.mult, op1=OP.add, accum_out=bzc[:])
                nc.vector.tensor_tensor_reduce(
                    out=junk[:], in0=gt[:], in1=pqb_sb[:], scale=1.0, scalar=float(c_s0),
                    op0=OP.mult, op1=OP.add, accum_out=s0z[:])
            else:
                bz0 = sp.tile([PT, 1], FP32, tag="bz0")
                nc.vector.tensor_mul(out=junk[:], in0=gt[:], in1=pb_sb[:])
                nc.vector.reduce_sum(out=bz0[:], in_=junk[:], axis=AX.X)
                nc.scalar.activation(out=bzc[:], in_=bz0[:], func=AF.Identity,
                                     bias=czt[:, 0:1])
                nc.vector.tensor_mul(out=junk[:], in0=gt[:], in1=pqb_sb[:])
                nc.vector.reduce_sum(out=bz0[:], in_=junk[:], axis=AX.X)
                nc.scalar.activation(out=s0z[:], in_=bz0[:], func=AF.Identity,
                                     bias=cst[:, 0:1])
            s0 = sp.tile([PT, 1], FP32, tag="s0")
            nc.scalar.activation(out=s0[:], in_=s0z[:], func=AF.Lrelu, alpha=0.01)

            # ---- t-pass: tree-fold ymv over d into t16 ----
            td = tdp.tile([PT, L, LD // 2], BF16, tag="td")
            if it == 0:
                # split L1 per chunk: overlaps the initial conversions
                for cchunk in range(NCH):
                    ls = slice(cchunk * LCH, (cchunk + 1) * LCH)
                    nc.vector.tensor_add(out=td[:, ls, :],
                                         in0=ymv[:, ls, 0:LD // 2],
                                         in1=ymv[:, ls, LD // 2:LD])
            else:
                nc.vector.tensor_add(out=td[:], in0=ymv[:, :, 0:LD // 2],
                                     in1=ymv[:, :, LD // 2:LD])
            gp_eng = nc.gpsimd if GP_TAILS else nc.vector
            dh = LD // 2
            lvl = 1
            while dh > 2:
                h = dh // 2
                eng = nc.vector if lvl < DVE_TREE_LEVELS else gp_eng
                eng.tensor_add(out=td[:, :, 0:h], in0=td[:, :, 0:h],
                               in1=td[:, :, h:dh])
                dh = h
                lvl += 1
            t16 = sp.tile([PT, L], FP32, tag="t16")
            gp_eng.tensor_add(
                out=t16[:].rearrange("p (l o) -> p l o", o=1),
                in0=td[:, :, 0:1], in1=td[:, :, 1:2])

            # ---- software pipelining: enqueue next tile's load NOW, so the
            # Act queue does conversions (and DVE the v-mults) while this
            # tile's t16 -> sl -> slb2 cross-engine round-trip is in flight.
            if it + 1 < NT:
                loaded = emit_load(it + 1)

            # sl = lrelu(t + bzc); S = sum(sl)  (single fused Act op, fp32 in
            # -> bf16 out so the slb2 broadcast copy is a bf16->bf16 pattern)
            mS = sp.tile([PT, LD + 1], FP32, tag="mS")
            sl = sp.tile([PT, L], BF16, tag="sl")
            nc.scalar.activation(
                out=sl[:], in_=t16[:], func=AF.Lrelu,
                bias=bzc[:], alpha=0.01, accum_out=mS[:, LD:LD + 1])

            if it == 0:
                emit_mlp_weight_loads()

            # slb2: sl duplicated pairs in bf16 [P, L, 2]
            slb2 = sp.tile([PT, L, 2], BF16, tag="slb2")
            nc.vector.tensor_copy(
                out=slb2[:],
                in_=sl[:].rearrange("p (l o) -> p l o", o=1).to_broadcast([PT, L, 2]))

            # ---- m-pass: ym = ymv * sl, tree-fold over l ----
            if INPLACE_YM:
                ymt = ymv
            else:
                ymt = ymbp.tile([PT, L, LD], BF16, tag="ymb")
            ymi = ymv[:].rearrange("p l (o t) -> p l o t", o=LD // 2)
            ymo = ymt[:].rearrange("p l (o t) -> p l o t", o=LD // 2)
            nc.vector.tensor_mul(
                out=ymo, in0=ymi,
                in1=slb2[:].rearrange("p l (o t) -> p l o t", o=1)
                    .to_broadcast([PT, L, LD // 2, 2]))
            cur = ymt[:]
            n = L
            lvl = 0
            while n > 3:
                h = n // 2
                odd = n - 2 * h
                eng = nc.vector if lvl < DVE_TREE_LEVELS else gp_eng
                eng.tensor_add(out=cur[:, 0:h, :], in0=cur[:, 0:h, :],
                               in1=cur[:, h:2 * h, :])
                if odd:
                    eng.tensor_add(out=cur[:, 0:1, :], in0=cur[:, 0:1, :],
                                   in1=cur[:, 2 * h:n, :])
                n = h
                lvl += 1
            # final level(s) -> fp32 m' into mS[:, 0:64]
            m1 = mS[:, 0:LD].rearrange("p (o d) -> p o d", o=1)
            gp_eng.tensor_add(out=m1, in0=cur[:, 0:1, :], in1=cur[:, 1:2, :])
            if n == 3:
                gp_eng.tensor_add(out=m1, in0=m1, in1=cur[:, 2:3, :])

            # ---- normalization ----
            total = sp.tile([PT, 1], FP32, tag="total")
            nc.vector.tensor_add(out=total[:], in0=s0[:], in1=mS[:, LD:LD + 1])
            rc = sp.tile([PT, 1], FP32, tag="rc")
            nc.vector.reciprocal(out=rc[:], in_=total[:])

            # g_aug = [gt, 1] * n0s ; mSs = mS * rinv * rc
            g_aug = sp.tile([PT, GD + 1], FP32, tag="g_aug")
            gp_eng.tensor_mul(out=g_aug[:, GD:GD + 1], in0=s0[:], in1=rc[:])
            gp_eng.tensor_mul(
                out=g_aug[:, 0:GD], in0=gt[:],
                in1=g_aug[:, GD:GD + 1].to_broadcast([PT, GD]))
            mSs = sp.tile([PT, LD + 1], FP32, tag="mSs")
            gp_eng.tensor_mul(out=mSs[:], in0=mS[:], in1=rinv_sb[:])
            gp_eng.tensor_mul(out=mSs[:], in0=mSs[:],
                              in1=rc[:].to_broadcast([PT, LD + 1]))

            # ---- transposes + phase-A matmuls (bf16) ----
            pG = psA.tile([GD + 1, PT], FP32, tag="pG")
            nc.tensor.transpose(pG[:], g_aug[:], ident[:])
            gTs = fp.tile([GD + 1, PT], BF16, tag="gTs")
            nc.scalar.copy(out=gTs[:], in_=pG[:])

            pM = psA.tile([LD + 1, PT], FP32, tag="pG")
            nc.tensor.transpose(pM[:], mSs[:], ident[:])
            msts = fp.tile([LD + 1, PT], BF16, tag="msts")
            nc.scalar.copy(out=msts[:], in_=pM[:])

            sh = states_halves[it // 2]
            c0 = (it % 2) * PT
            pW = psB.tile([H, PT], FP32, tag="pW")
            nc.tensor.matmul(pW[:], lhsT=waug_sb[:], rhs=gTs[:], start=True, stop=True)
            nc.scalar.activation(out=sh[0:H, c0:c0 + PT], in_=pW[:], func=AF.Relu)
            pAg = psB.tile([H, PT], FP32, tag="pW")
            nc.tensor.matmul(pAg[:], lhsT=uaug_sb[:], rhs=msts[:], start=True, stop=True)
            nc.scalar.activation(out=sh[H:2 * H, c0:c0 + PT], in_=pAg[:], func=AF.Relu)

            emit_mlp(it)

    nc.compile()
    return nc


def _prep(inputs):
    import ml_dtypes
    W_w = np.asarray(inputs["W_w"], np.float32)
    W_b = np.asarray(inputs["W_b"], np.float32)
    U_w = np.asarray(inputs["U_w"], np.float32)
    U_b = np.asarray(inputs["U_b"], np.float32)
    att_w = np.asarray(inputs["att_w"], np.float32)
    att_b = np.asarray(inputs["att_b"], np.float32)
    l1_w = np.asarray(inputs["l1_w"], np.float32)
    l1_b = np.asarray(inputs["l1_b"], np.float32)
    l2_w = np.asarray(inputs["l2_w"], np.float32)
    l2_b = np.asarray(inputs["l2_b"], np.float32)
    l3_w = np.asarray(inputs["l3_w"], np.float32)
    l3_b = np.asarray(inputs["l3_b"], np.float32)

    a_g, a_l = att_w[0, :H], att_w[0, H:]
    v = U_w.T @ a_l
    p = W_w.T @ a_g
    q = W_w.T @ a_l
    c_g = float(a_g @ W_b)
    c_q = float(a_l @ W_b)
    c_v = float(a_l @ U_b)
    ab = float(att_b[0])
    c_z = c_g + c_v + ab
    c_s0 = c_g + c_q + ab

    bf = ml_dtypes.bfloat16
    # 1/v computed against the bf16-rounded v actually used on device
    v16 = v.astype(bf)
    v16f = v16.astype(np.float32)
    v16f = np.where(np.abs(v16f) < 1e-20, 1e-20, v16f)
    rinv_row = np.concatenate([1.0 / v16f, [1.0]]).astype(np.float32)

    consts = dict(
        waug=np.ascontiguousarray(np.vstack([W_w.T, W_b[None, :]]).astype(bf)),
        uaug=np.ascontiguousarray(np.vstack([U_w.T, U_b[None, :]]).astype(bf)),
        vb16=np.ascontiguousarray(np.broadcast_to(v16, (128, LD))),
        rinv=np.ascontiguousarray(np.broadcast_to(rinv_row, (128, LD + 1))),
        pb=np.ascontiguousarray(np.broadcast_to(p, (128, GD)).astype(np.float32)),
        pqb=np.ascontiguousarray(np.broadcast_to(p + q, (128, GD)).astype(np.float32)),
        l1wt=np.ascontiguousarray(l1_w.T.astype(bf)),
        l1b=np.ascontiguousarray(l1_b.reshape(2, 128).T.astype(np.float32)),
        l2wt=np.ascontiguousarray(l2_w.T.astype(bf)),
        l2b=np.ascontiguousarray(l2_b.reshape(2, 128).T.astype(np.float32)),
        l3wt=np.ascontiguousarray(l3_w.T.astype(bf)),
        l3b=np.ascontiguousarray(l3_b[:, None].astype(np.float32)),
    )
    return consts, c_z, c_s0


def _get_graph_and_consts(inputs):
    consts, c_z, c_s0 = _prep(inputs)
    key = (c_z, c_s0)
    if key not in _CACHE:
        _CACHE[key] = build_graph(c_z, c_s0)
    return _CACHE[key], consts


def kernel(**inputs) -> np.ndarray:
    nc, consts = _get_graph_and_consts(inputs)
    gs = np.ascontiguousarray(np.asarray(inputs["global_states"], np.float32))
    ls = np.ascontiguousarray(np.asarray(inputs["local_states"], np.float32))
    in_maps = []
    for i in range(NCORES):
        m = dict(consts)
        m["x"] = np.ascontiguousarray(ls[i * BC:(i + 1) * BC])
        m["g"] = np.ascontiguousarray(gs[i * BC:(i + 1) * BC])
        in_maps.append(m)
    res = run_bass_kernel_spmd(nc, in_maps, list(range(NCORES)))
    outs = [res.results[i]["out"] for i in range(NCORES)]
    return np.concatenate(outs, axis=0).astype(np.float32)


# revision 7
# speedup vs baseline: 1.1933x; 1.0033x over previous
"""Trainium2 Bass kernel for nn_Actor (GNN message-passing actor network), v2.

Math (per sample b):
  v  = U_w.T @ a_l ; p = W_w.T @ a_g ; q = W_w.T @ a_l
  c_z  = a_g.W_b + a_l.U_b + att_b ; c_s0 = a_g.W_b + a_l.W_b + att_b
  ymv[b,l,d] = x[b,l,d] * v[d]                (bf16)
  t[b,l]  = sum_d ymv                         (tensor_reduce over d)
  sl      = lrelu(t + G.p + c_z);  s0 = lrelu(G.(p+q) + c_s0)
  total   = s0 + sum sl;  S = sum sl
  m'[b,d] = sum_l sl*ymv = v[d] * sum_l sl*x  (bf16 mult + in-place tree)
  mSs     = [m' * (1/v), S] / total ; g_aug = [G, 1]*s0/total
  states  = relu([ [W_w.T; W_b]^T g_aug ; [U_w.T; U_b]^T mSs ])
  out     = sigmoid(l3(relu(l2(relu(l1(states))))))

Sharding: pure data parallel, batch 4096 -> 8 cores x 512.
Engine split: Act converts x->bf16 + all activation/bias ops; DVE does the two
big bf16 multiplies + reduce + tree top; GpSimd does tree tail + small scaling;
PE does transposes + all matmuls in bf16.
"""

import numpy as np
from contextlib import ExitStack

import concourse.bass as bass
import concourse.bacc as bacc
import concourse.tile as tile
from concourse import masks, mybir
from concourse.bass_utils import run_bass_kernel_spmd

FP32 = mybir.dt.float32
BF16 = mybir.dt.bfloat16
AX = mybir.AxisListType
OP = mybir.AluOpType
AF = mybir.ActivationFunctionType

B, L = 4096, 200
GD, LD, AD, H = 64, 64, 8, 32
NCORES = 8
BC = B // NCORES          # 512 samples per core
PT = 128                  # samples per tile
NT = BC // PT             # 4 tiles per core

# how many tree levels stay on DVE before handing to GpSimd (rest of levels)
DVE_TREE_LEVELS = 2
USE_TTR = False      # tensor_tensor_reduce for the G-dot products
GP_TAILS = False     # gpsimd handles tree tails + small scaling ops
INPLACE_YM = True    # m-pass multiply in place on ymv

_CACHE = {}


def build_graph(c_z: float, c_s0: float):
    nc = bacc.Bacc()
    x = nc.declare_dram_parameter("x", [BC, L, LD], FP32, isOutput=False)
    g = nc.declare_dram_parameter("g", [BC, GD], FP32, isOutput=False)
    waug = nc.declare_dram_parameter("waug", [GD + 1, H], BF16, isOutput=False)
    uaug = nc.declare_dram_parameter("uaug", [LD + 1, H], BF16, isOutput=False)
    vb16 = nc.declare_dram_parameter("vb16", [128, LD], BF16, isOutput=False)
    rinv = nc.declare_dram_parameter("rinv", [128, LD + 1], FP32, isOutput=False)
    pb = nc.declare_dram_parameter("pb", [128, GD], FP32, isOutput=False)
    pqb = nc.declare_dram_parameter("pqb", [128, GD], FP32, isOutput=False)
    l1wt = nc.declare_dram_parameter("l1wt", [GD, 256], BF16, isOutput=False)
    l1b = nc.declare_dram_parameter("l1b", [128, 2], FP32, isOutput=False)
    l2wt = nc.declare_dram_parameter("l2wt", [256, 256], BF16, isOutput=False)
    l2b = nc.declare_dram_parameter("l2b", [128, 2], FP32, isOutput=False)
    l3wt = nc.declare_dram_parameter("l3wt", [256, AD], BF16, isOutput=False)
    l3b = nc.declare_dram_parameter("l3b", [AD, 1], FP32, isOutput=False)
    out = nc.declare_dram_parameter("out", [BC, AD], FP32, isOutput=True)

    with tile.TileContext(nc) as tc, ExitStack() as ctx:
        consts = ctx.enter_context(tc.tile_pool(name="consts", bufs=1))

        ident = consts.tile([128, 128], FP32)
        masks.make_identity(nc, ident[:])
        czt = consts.tile([128, 1], FP32)
        nc.vector.memset(czt[:], float(c_z))
        cst = consts.tile([128, 1], FP32)
        nc.vector.memset(cst[:], float(c_s0))

        waug_sb = consts.tile([GD + 1, H], BF16)
        nc.scalar.dma_start(out=waug_sb[:], in_=waug[:])
        uaug_sb = consts.tile([LD + 1, H], BF16)
        nc.scalar.dma_start(out=uaug_sb[:], in_=uaug[:])
        vb_sb = consts.tile([128, 1, LD], BF16)
        nc.sync.dma_start(out=vb_sb[:], in_=vb16[:].rearrange("p (o d) -> p o d", o=1))
        rinv_sb = consts.tile([128, LD + 1], FP32)
        nc.scalar.dma_start(out=rinv_sb[:], in_=rinv[:])
        pb_sb = consts.tile([128, GD], FP32)
        nc.scalar.dma_start(out=pb_sb[:], in_=pb[:])
        pqb_sb = consts.tile([128, GD], FP32)
        nc.scalar.dma_start(out=pqb_sb[:], in_=pqb[:])
        l1wt_sb = consts.tile([GD, 256], BF16)
        l1b_sb = consts.tile([128, 2], FP32)
        l2wt_a = consts.tile([128, 256], BF16)
        l2wt_b = consts.tile([128, 256], BF16)
        l2b_sb = consts.tile([128, 2], FP32)
        l3wt_a = consts.tile([128, AD], BF16)
        l3wt_b = consts.tile([128, AD], BF16)
        l3b_sb = consts.tile([AD, 1], FP32)

        def emit_mlp_weight_loads():
            # deferred so the Act sequencer isn't stuck issuing DMAs while
            # tile-0 conversions could run; first consumer is the MLP (~40us)
            nc.scalar.dma_start(out=l1wt_sb[:], in_=l1wt[:])
            nc.scalar.dma_start(out=l1b_sb[:], in_=l1b[:])
            nc.scalar.dma_start(out=l2wt_a[:], in_=l2wt[0:128])
            nc.scalar.dma_start(out=l2wt_b[:], in_=l2wt[128:256])
            nc.scalar.dma_start(out=l2b_sb[:], in_=l2b[:])
            nc.scalar.dma_start(out=l3wt_a[:], in_=l3wt[0:128])
            nc.scalar.dma_start(out=l3wt_b[:], in_=l3wt[128:256])
            nc.scalar.dma_start(out=l3b_sb[:], in_=l3b[:])

        NCH = 4                    # x DMA/convert chunks per tile
        LCH = L // NCH
        xfp = ctx.enter_context(tc.tile_pool(name="xfp", bufs=4))
        xf0p = ctx.enter_context(tc.tile_pool(name="xf0p", bufs=4))
        xb0p = ctx.enter_context(tc.tile_pool(name="xb0p", bufs=4))
        xbp = ctx.enter_context(tc.tile_pool(name="xbp", bufs=4))
        ymp = ctx.enter_context(tc.tile_pool(name="ymp", bufs=2))
        ymbp = ctx.enter_context(tc.tile_pool(name="ymbp", bufs=1))
        tdp = ctx.enter_context(tc.tile_pool(name="tdp", bufs=1))
        sp = ctx.enter_context(tc.tile_pool(name="sp", bufs=2))
        fp = ctx.enter_context(tc.tile_pool(name="fp", bufs=2))
        glob = ctx.enter_context(tc.tile_pool(name="glob", bufs=1))
        psA = ctx.enter_context(tc.tile_pool(name="psA", bufs=2, space="PSUM"))
        psB = ctx.enter_context(tc.tile_pool(name="psB", bufs=2, space="PSUM"))
        ps1 = ctx.enter_context(tc.tile_pool(name="ps1", bufs=1, space="PSUM"))

        st_h0 = glob.tile([2 * H, BC // 2], BF16, tag="st_h0")
        st_h1 = glob.tile([2 * H, BC // 2], BF16, tag="st_h1")
        states_halves = [st_h0, st_h1]

        BH = PT

        def emit_mlp(it):
            """MLP for one tile's 128 samples; emitted right after the tile's
            states columns complete so it overlaps later tiles' streaming."""
            str_ = states_halves[it // 2][:, (it % 2) * PT:(it % 2) * PT + PT]

            p1a = ps1.tile([128, BH], FP32, tag="p1a")
            nc.tensor.matmul(p1a[:], lhsT=l1wt_sb[:, 0:128], rhs=str_, start=True, stop=True)
            p1b = ps1.tile([128, BH], FP32, tag="p1b")
            nc.tensor.matmul(p1b[:], lhsT=l1wt_sb[:, 128:256], rhs=str_, start=True, stop=True)
            a1a = fp.tile([128, BH], BF16, tag="a1a")
            nc.scalar.activation(out=a1a[:], in_=p1a[:], func=AF.Relu,
                                 bias=l1b_sb[:, 0:1], scale=1.0)
            a1b = fp.tile([128, BH], BF16, tag="a1b")
            nc.scalar.activation(out=a1b[:], in_=p1b[:], func=AF.Relu,
                                 bias=l1b_sb[:, 1:2], scale=1.0)

            p2a = ps1.tile([128, BH], FP32, tag="p1a")
            nc.tensor.matmul(p2a[:], lhsT=l2wt_a[:, 0:128], rhs=a1a[:], start=True, stop=False)
            nc.tensor.matmul(p2a[:], lhsT=l2wt_b[:, 0:128], rhs=a1b[:], start=False, stop=True)
            p2b = ps1.tile([128, BH], FP32, tag="p1b")
            nc.tensor.matmul(p2b[:], lhsT=l2wt_a[:, 128:256], rhs=a1a[:], start=True, stop=False)
            nc.tensor.matmul(p2b[:], lhsT=l2wt_b[:, 128:256], rhs=a1b[:], start=False, stop=True)
            a2a = fp.tile([128, BH], BF16, tag="a2a")
            nc.scalar.activation(out=a2a[:], in_=p2a[:], func=AF.Relu,
                                 bias=l2b_sb[:, 0:1], scale=1.0)
            a2b = fp.tile([128, BH], BF16, tag="a2b")
            nc.scalar.activation(out=a2b[:], in_=p2b[:], func=AF.Relu,
                                 bias=l2b_sb[:, 1:2], scale=1.0)

            p3 = ps1.tile([AD, BH], FP32, tag="p1a")
            nc.tensor.matmul(p3[:], lhsT=l3wt_a[:], rhs=a2a[:], start=True, stop=False)
            nc.tensor.matmul(p3[:], lhsT=l3wt_b[:], rhs=a2b[:], start=False, stop=True)
            oT = fp.tile([AD, BH], FP32, tag="oT")
            nc.scalar.activation(out=oT[:], in_=p3[:], func=AF.Sigmoid,
                                 bias=l3b_sb[:], scale=1.0)

            b0o = it * PT
            pO = psA.tile([PT, AD], FP32, tag="pO")
            nc.tensor.transpose(pO[:], oT[:], ident[0:AD, 0:AD])
            ob = fp.tile([PT, AD], FP32, tag="ob")
            nc.scalar.copy(out=ob[:], in_=pO[:])
            nc.sync.dma_start(out=out[b0o:b0o + PT, :], in_=ob[:])

        def emit_load(it):
            """DMA x in chunks; convert fp32->bf16 on Act; multiply by v on
            DVE per chunk. Tile 0 uses 8 small chunks so the first
            conversion starts before the DMA clock has ramped up.
            Returns (ymv, gt) for the tile."""
            b0 = it * PT
            ymv = ymp.tile([PT, L, LD], BF16, tag="ymv")
            nch = 8 if it == 0 else NCH
            lch = L // nch
            for cchunk in range(nch):
                ls = slice(cchunk * lch, (cchunk + 1) * lch)
                if it == 0:
                    xf = xf0p.tile([PT, lch, LD], FP32, tag="xf0")
                    xbh = xb0p.tile([PT, lch, LD], BF16, tag="xbh0")
                else:
                    xf = xfp.tile([PT, lch, LD], FP32, tag="xf")
                    xbh = xbp.tile([PT, lch, LD], BF16, tag="xbh")
                nc.sync.dma_start(out=xf[:], in_=x[b0:b0 + PT, ls])
                nc.scalar.copy(out=xbh[:], in_=xf[:])
                nc.vector.tensor_mul(
                    out=ymv[:, ls, :], in0=xbh[:],
                    in1=vb_sb[:].to_broadcast([PT, lch, LD]))
            gt = sp.tile([PT, GD], FP32, tag="gt")
            nc.sync.dma_start(out=gt[:], in_=g[b0:b0 + PT])
            return ymv, gt

        loaded = emit_load(0)
        for it in range(NT):
            b0 = it * PT
            ymv, gt = loaded

            # ---- per-sample bias dots ----
            junk = sp.tile([PT, GD], FP32, tag="junk")
            bzc = sp.tile([PT, 1], FP32, tag="bzc")
            s0z = sp.tile([PT, 1], FP32, tag="s0z")
            if USE_TTR:
                nc.vector.tensor_tensor_reduce(
                    out=junk[:], in0=gt[:], in1=pb_sb[:], scale=1.0, scalar=float(c_z),
                    op0=OP.mult, op1=OP.add, accum_out=bzc[:])
                nc.vector.tensor_tensor_reduce(
                    out=junk[:], in0=gt[:], in1=pqb_sb[:], scale=1.0, scalar=float(c_s0),
                    op0=OP.mult, op1=OP.add, accum_out=s0z[:])
            else:
                bz0 = sp.tile([PT, 1], FP32, tag="bz0")
                junk2 = sp.tile([PT, GD], FP32, tag="junk2")
                # multiplies on idle GpSimd (off the critical path: results
                # are only needed by the Act sl op ~8us later)
                nc.gpsimd.tensor_mul(out=junk[:], in0=gt[:], in1=pb_sb[:])
                nc.gpsimd.tensor_mul(out=junk2[:], in0=gt[:], in1=pqb_sb[:])
                nc.vector.reduce_sum(out=bz0[:], in_=junk[:], axis=AX.X)
                nc.scalar.activation(out=bzc[:], in_=bz0[:], func=AF.Identity,
                                     bias=czt[:, 0:1])
                nc.vector.reduce_sum(out=bz0[:], in_=junk2[:], axis=AX.X)
                nc.scalar.activation(out=s0z[:], in_=bz0[:], func=AF.Identity,
                                     bias=cst[:, 0:1])
            s0 = sp.tile([PT, 1], FP32, tag="s0")
            nc.scalar.activation(out=s0[:], in_=s0z[:], func=AF.Lrelu, alpha=0.01)

            # ---- t-pass: tree-fold ymv over d into t16 ----
            td = tdp.tile([PT, L, LD // 2], BF16, tag="td")
            if it == 0:
                # split L1 per chunk: overlaps the initial conversions
                for cchunk in range(NCH):
                    ls = slice(cchunk * LCH, (cchunk + 1) * LCH)
                    nc.vector.tensor_add(out=td[:, ls, :],
                                         in0=ymv[:, ls, 0:LD // 2],
                                         in1=ymv[:, ls, LD // 2:LD])
            else:
                nc.vector.tensor_add(out=td[:], in0=ymv[:, :, 0:LD // 2],
                                     in1=ymv[:, :, LD // 2:LD])
            gp_eng = nc.gpsimd if GP_TAILS else nc.vector
            dh = LD // 2
            lvl = 1
            while dh > 2:
                h = dh // 2
                eng = nc.vector if lvl < DVE_TREE_LEVELS else gp_eng
                eng.tensor_add(out=td[:, :, 0:h], in0=td[:, :, 0:h],
                               in1=td[:, :, h:dh])
                dh = h
                lvl += 1
            t16 = sp.tile([PT, L], FP32, tag="t16")
            gp_eng.tensor_add(
                out=t16[:].rearrange("p (l o) -> p l o", o=1),
                in0=td[:, :, 0:1], in1=td[:, :, 1:2])

            # ---- software pipelining: enqueue next tile's load NOW, so the
            # Act queue does conversions (and DVE the v-mults) while this
            # tile's t16 -> sl -> slb2 cross-engine round-trip is in flight.
            if it + 1 < NT:
                loaded = emit_load(it + 1)

            # sl = lrelu(t + bzc); S = sum(sl)  (single fused Act op, fp32 in
            # -> bf16 out so the slb2 broadcast copy is a bf16->bf16 pattern)
            mS = sp.tile([PT, LD + 1], FP32, tag="mS")
            sl = sp.tile([PT, L], BF16, tag="sl")
            nc.scalar.activation(
                out=sl[:], in_=t16[:], func=AF.Lrelu,
                bias=bzc[:], alpha=0.01, accum_out=mS[:, LD:LD + 1])

            if it == 0:
                emit_mlp_weight_loads()

            # slb2: sl duplicated pairs in bf16 [P, L, 2]
            slb2 = sp.tile([PT, L, 2], BF16, tag="slb2")
            nc.vector.tensor_copy(
                out=slb2[:],
                in_=sl[:].rearrange("p (l o) -> p l o", o=1).to_broadcast([PT, L, 2]))

            # ---- m-pass: ym = ymv * sl, tree-fold over l ----
            if INPLACE_YM:
                ymt = ymv
            else:
                ymt = ymbp.tile([PT, L, LD], BF16, tag="ymb")
            ymi = ymv[:].rearrange("p l (o t) -> p l o t", o=LD // 2)
            ymo = ymt[:].rearrange("p l (o t) -> p l o t", o=LD // 2)
            nc.vector.tensor_mul(
                out=ymo, in0=ymi,
                in1=slb2[:].rearrange("p l (o t) -> p l o t", o=1)
                    .to_broadcast([PT, L, LD // 2, 2]))
            cur = ymt[:]
            n = L
            lvl = 0
            while n > 3:
                h = n // 2
                odd = n - 2 * h
                eng = nc.vector if lvl < DVE_TREE_LEVELS else gp_eng
                eng.tensor_add(out=cur[:, 0:h, :], in0=cur[:, 0:h, :],
                               in1=cur[:, h:2 * h, :])
                if odd:
                    eng.tensor_add(out=cur[:, 0:1, :], in0=cur[:, 0:1, :],
                                   in1=cur[:, 2 * h:n, :])
                n = h
                lvl += 1
            # final level(s) -> fp32 m' into mS[:, 0:64]
            m1 = mS[:, 0:LD].rearrange("p (o d) -> p o d", o=1)
            gp_eng.tensor_add(out=m1, in0=cur[:, 0:1, :], in1=cur[:, 1:2, :])
            if n == 3:
                gp_eng.tensor_add(out=m1, in0=m1, in1=cur[:, 2:3, :])

            # ---- normalization ----
            total = sp.tile([PT, 1], FP32, tag="total")
            nc.vector.tensor_add(out=total[:], in0=s0[:], in1=mS[:, LD:LD + 1])
            rc = sp.tile([PT, 1], FP32, tag="rc")
            nc.vector.reciprocal(out=rc[:], in_=total[:])

            # g_aug = [gt, 1] * n0s ; mSs = mS * rinv * rc
            g_aug = sp.tile([PT, GD + 1], FP32, tag="g_aug")
            gp_eng.tensor_mul(out=g_aug[:, GD:GD + 1], in0=s0[:], in1=rc[:])
            gp_eng.tensor_mul(
                out=g_aug[:, 0:GD], in0=gt[:],
                in1=g_aug[:, GD:GD + 1].to_broadcast([PT, GD]))
            mSs = sp.tile([PT, LD + 1], FP32, tag="mSs")
            gp_eng.tensor_mul(out=mSs[:], in0=mS[:], in1=rinv_sb[:])
            gp_eng.tensor_mul(out=mSs[:], in0=mSs[:],
                              in1=rc[:].to_broadcast([PT, LD + 1]))

            # ---- transposes + phase-A matmuls (bf16) ----
            pG = psA.tile([GD + 1, PT], FP32, tag="pG")
            nc.tensor.transpose(pG[:], g_aug[:], ident[:])
            gTs = fp.tile([GD + 1, PT], BF16, tag="gTs")
            nc.scalar.copy(out=gTs[:], in_=pG[:])

            pM = psA.tile([LD + 1, PT], FP32, tag="pG")
            nc.tensor.transpose(pM[:], mSs[:], ident[:])
            msts = fp.tile([LD + 1, PT], BF16, tag="msts")
            nc.scalar.copy(out=msts[:], in_=pM[:])

            sh = states_halves[it // 2]
            c0 = (it % 2) * PT
            pW = psB.tile([H, PT], FP32, tag="pW")
            nc.tensor.matmul(pW[:], lhsT=waug_sb[:], rhs=gTs[:], start=True, stop=True)
            nc.scalar.activation(out=sh[0:H, c0:c0 + PT], in_=pW[:], func=AF.Relu)
            pAg = psB.tile([H, PT], FP32, tag="pW")
            nc.tensor.matmul(pAg[:], lhsT=uaug_sb[:], rhs=msts[:], start=True, stop=True)
            nc.scalar.activation(out=sh[H:2 * H, c0:c0 + PT], in_=pAg[:], func=AF.Relu)

            emit_mlp(it)

    nc.compile()
    return nc


def _prep(inputs):
    import ml_dtypes
    W_w = np.asarray(inputs["W_w"], np.float32)
    W_b = np.asarray(inputs["W_b"], np.float32)
    U_w = np.asarray(inputs["U_w"], np.float32)
    U_b = np.asarray(inputs["U_b"], np.float32)
    att_w = np.asarray(inputs["att_w"], np.float32)
    att_b = np.asarray(inputs["att_b"], np.float32)
    l1_w = np.asarray(inputs["l1_w"], np.float32)
    l1_b = np.asarray(inputs["l1_b"], np.float32)
    l2_w = np.asarray(inputs["l2_w"], np.float32)
    l2_b = np.asarray(inputs["l2_b"], np.float32)
    l3_w = np.asarray(inputs["l3_w"], np.float32)
    l3_b = np.asarray(inputs["l3_b"], np.float32)

    a_g, a_l = att_w[0, :H], att_w[0, H:]
    v = U_w.T @ a_l
    p = W_w.T @ a_g
    q = W_w.T @ a_l
    c_g = float(a_g @ W_b)
    c_q = float(a_l @ W_b)
    c_v = float(a_l @ U_b)
    ab = float(att_b[0])
    c_z = c_g + c_v + ab
    c_s0 = c_g + c_q + ab

    bf = ml_dtypes.bfloat16
    # 1/v computed against the bf16-rounded v actually used on device
    v16 = v.astype(bf)
    v16f = v16.astype(np.float32)
    v16f = np.where(np.abs(v16f) < 1e-20, 1e-20, v16f)
    rinv_row = np.concatenate([1.0 / v16f, [1.0]]).astype(np.float32)

    consts = dict(
        waug=np.ascontiguousarray(np.vstack([W_w.T, W_b[None, :]]).astype(bf)),
        uaug=np.ascontiguousarray(np.vstack([U_w.T, U_b[None, :]]).astype(bf)),
        vb16=np.ascontiguousarray(np.broadcast_to(v16, (128, LD))),
        rinv=np.ascontiguousarray(np.broadcast_to(rinv_row, (128, LD + 1))),
        pb=np.ascontiguousarray(np.broadcast_to(p, (128, GD)).astype(np.float32)),
        pqb=np.ascontiguousarray(np.broadcast_to(p + q, (128, GD)).astype(np.float32)),
        l1wt=np.ascontiguousarray(l1_w.T.astype(bf)),
        l1b=np.ascontiguousarray(l1_b.reshape(2, 128).T.astype(np.float32)),
        l2wt=np.ascontiguousarray(l2_w.T.astype(bf)),
        l2b=np.ascontiguousarray(l2_b.reshape(2, 128).T.astype(np.float32)),
        l3wt=np.ascontiguousarray(l3_w.T.astype(bf)),
        l3b=np.ascontiguousarray(l3_b[:, None].astype(np.float32)),
    )
    return consts, c_z, c_s0


def _get_graph_and_consts(inputs):
    consts, c_z, c_s0 = _prep(inputs)
    key = (c_z, c_s0)
    if key not in _CACHE:
        _CACHE[key] = build_graph(c_z, c_s0)
    return _CACHE[key], consts


def kernel(**inputs) -> np.ndarray:
    nc, consts = _get_graph_and_consts(inputs)
    gs = np.ascontiguousarray(np.asarray(inputs["global_states"], np.float32))
    ls = np.ascontiguousarray(np.asarray(inputs["local_states"], np.float32))
    in_maps = []
    for i in range(NCORES):
        m = dict(consts)
        m["x"] = np.ascontiguousarray(ls[i * BC:(i + 1) * BC])
        m["g"] = np.ascontiguousarray(gs[i * BC:(i + 1) * BC])
        in_maps.append(m)
    res = run_bass_kernel_spmd(nc, in_maps, list(range(NCORES)))
    outs = [res.results[i]["out"] for i in range(NCORES)]
    return np.concatenate(outs, axis=0).astype(np.float32)
